# revision 33
# baseline (speedup 1.0000x reference)
"""Trainium2 Bass kernel for nn_DiscreteFullyConnectedQFunction (GIN message passing + dueling Q heads).

Strategy: data-parallel over batch (8 graphs per core, 8 cores). All heavy
matmuls in fp16 at full PE rate (T-layout, feature dim on partitions, adjT
streamed as the moving operand); the full 16MB fp16 adjT stays SBUF-resident
so it is read from HBM exactly once, prefetched under the early phases.
Layer-1 pooling (DIN=2, 1.6% of FLOPs) is done on host in fp32 so the adjT
load never gates the first BN sync. Exact training-mode BatchNorm via four
tiny cross-core AllGathers of per-feature (sum, sumsq) — the [128,2] payload
is byte-reinterpreted as [16,16] so the 8-rank concat fits 128 partitions,
then reduced locally (AG has ~2x lower latency than AllReduce). Candidate
gather and graph pooling are folded into one matmul against a host-built
one-hot/pool matrix. Linear biases inside the GIN blocks cancel exactly
under BatchNorm and are dropped. Heads run in fp32.
"""
import numpy as np
from contextlib import ExitStack

import concourse.bass as bass
import concourse.bacc as bacc
import concourse.tile as tile
import concourse.mybir as mybir
from concourse.bass_utils import run_bass_kernel_spmd
from concourse.masks import make_identity

N_CORES = 8
B, NN, NJ = 64, 1000, 50
GPC = B // N_CORES          # graphs per core
H, HC = 128, 256
NT = 8                      # node tiles of 128 (1000 padded to 1024)
NPAD = NT * 128
BN_EPS = 1e-5
ROWS_CORE = float(GPC * NN)  # BN rows per core
ROWS_TOT = float(B * NN)     # BN rows globally
GCOLS = 52                  # 50 candidate one-hot cols + graph_pool + pad
CH = [(0, 512), (512, 488)]  # free-dim chunks, aligned to 2KB PSUM banks
SLAB = 4                    # adjT m-tiles per DMA
RES = 8                     # graphs whose adjT stays SBUF-resident for pass 2
F16 = mybir.dt.float16
F32 = mybir.dt.float32
AF = mybir.ActivationFunctionType
ALU = mybir.AluOpType
RG = [list(range(N_CORES))]

_CACHE = {}


def _build(debug_dump=False):
    nc = bacc.Bacc("TRN2", target_bir_lowering=False, debug=False, num_devices=N_CORES)
    dbg = {}
    if debug_dump:
        for nm, shape, dt in [
            ("dbg_p1", [2, NN], F16), ("dbg_a1", [H, 1], F32), ("dbg_c1", [H, 1], F32),
            ("dbg_y1", [H, NN], F16), ("dbg_h1nm", [128, NT * 128], F16),
            ("dbg_x2", [H, NN], F16), ("dbg_y2", [H, NN], F16),
            ("dbg_obs", [128, GPC * NJ], F32), ("dbg_hp", [128, GPC], F32),
            ("dbg_qrow", [1, GPC * NJ], F32), ("dbg_vrow", [1, GPC], F32),
            ("dbg_sqm", [1, GPC], F32), ("dbg_qsel", [1, GPC], F32),
        ]:
            dbg[nm] = nc.declare_dram_parameter(nm, shape, dt, isOutput=True)

    adjT = nc.declare_dram_parameter("adjT", [GPC, NPAD, NN], F16, isOutput=False)
    p1t = nc.declare_dram_parameter("p1t", [2, 2, GPC // 2 * NN], F16, isOutput=False)
    gmat = nc.declare_dram_parameter("gmat", [GPC, NT, 128, GCOLS], F16, isOutput=False)
    w1 = nc.declare_dram_parameter("w1", [2, H], F16, isOutput=False)
    w2 = nc.declare_dram_parameter("w2", [H, H], F16, isOutput=False)
    w1p = nc.declare_dram_parameter("w1p", [H, H], F16, isOutput=False)
    w2p = nc.declare_dram_parameter("w2p", [H, H], F16, isOutput=False)
    vecs1 = nc.declare_dram_parameter("vecs1", [H, 4], F32, isOutput=False)  # g_in,b_in,g_out,b_out
    vecs2 = nc.declare_dram_parameter("vecs2", [H, 4], F32, isOutput=False)
    wc1 = nc.declare_dram_parameter("wc1", [2 * H, HC], F32, isOutput=False)
    wc2 = nc.declare_dram_parameter("wc2", [HC, HC], F32, isOutput=False)
    wc3 = nc.declare_dram_parameter("wc3", [HC, 1], F32, isOutput=False)
    bc = nc.declare_dram_parameter("bc", [H, 4], F32, isOutput=False)  # b1c lo/hi, b2c lo/hi
    wv1 = nc.declare_dram_parameter("wv1", [H, HC], F32, isOutput=False)
    wv2 = nc.declare_dram_parameter("wv2", [HC, HC], F32, isOutput=False)
    wv3 = nc.declare_dram_parameter("wv3", [HC, 1], F32, isOutput=False)
    bv = nc.declare_dram_parameter("bv", [H, 4], F32, isOutput=False)
    maskr = nc.declare_dram_parameter("maskr", [1, GPC * NJ], F32, isOutput=False)
    ohr = nc.declare_dram_parameter("ohr", [1, GPC * NJ], F32, isOutput=False)
    minv = nc.declare_dram_parameter("minv", [1, GPC], F32, isOutput=False)
    scal = nc.declare_dram_parameter("scal", [1, 4], F32, isOutput=False)  # [b3v, ...]
    qout = nc.declare_dram_parameter("qout", [1, GPC], F32, isOutput=True)

    with tile.TileContext(nc) as tc, ExitStack() as ctx:
        const = ctx.enter_context(tc.tile_pool(name="const", bufs=1))
        small = ctx.enter_context(tc.tile_pool(name="small", bufs=1))
        dram = ctx.enter_context(tc.tile_pool(name="dram", bufs=1, space="DRAM"))
        statsp = ctx.enter_context(tc.tile_pool(name="stats", bufs=1))
        # one psum pool for the whole kernel: "mm" [128,1000] f32 x3 (6 banks)
        # + "tp" [128,128] f16 x2 (2 banks) = 8 banks
        psum = ctx.enter_context(tc.tile_pool(name="psum", bufs=1, space="PSUM"))

        # ---- constants / params in SBUF
        ident16 = const.tile([128, 128], F16, tag="ident")
        make_identity(nc, ident16[:])
        eps_t = const.tile([128, 1], F32, tag="eps")
        nc.vector.memset(eps_t[:], BN_EPS)

        def _load(name, ap, shape, dt, tag):
            t = const.tile(shape, dt, tag=tag, name=tag)
            nc.sync.dma_start(out=t[:], in_=ap)
            return t

        w1_sb = _load("w1", w1[:], [2, H], F16, "w1")
        w2_sb = _load("w2", w2[:], [H, H], F16, "w2")
        w1p_sb = _load("w1p", w1p[:], [H, H], F16, "w1p")
        w2p_sb = _load("w2p", w2p[:], [H, H], F16, "w2p")
        vecs1_sb = _load("vecs1", vecs1[:], [H, 4], F32, "vecs1")
        vecs2_sb = _load("vecs2", vecs2[:], [H, 4], F32, "vecs2")
        bc_sb = _load("bc", bc[:], [H, 4], F32, "bc")
        bv_sb = _load("bv", bv[:], [H, 4], F32, "bv")
        mask_sb = _load("maskr", maskr[:], [1, GPC * NJ], F32, "maskr")
        oh_sb = _load("ohr", ohr[:], [1, GPC * NJ], F32, "ohr")
        minv_sb = _load("minv", minv[:], [1, GPC], F32, "minv")
        scal_sb = _load("scal", scal[:], [1, 4], F32, "scal")


        stats_t = [statsp.tile([H, 2 * GPC, 6], F32, tag=f"stats{i}", name=f"stats{i}")
                   for i in range(4)]

        # ---- persistent activation pools
        p1sb_p = ctx.enter_context(tc.tile_pool(name="p1sb", bufs=2))
        y1sb_p = ctx.enter_context(tc.tile_pool(name="y1sb", bufs=GPC))
        h1nm_p = ctx.enter_context(tc.tile_pool(name="h1nm", bufs=2))
        sync2_p = ctx.enter_context(tc.tile_pool(name="sync2", bufs=GPC + 1))
        trans_p = ctx.enter_context(tc.tile_pool(name="trans", bufs=2))
        headp = ctx.enter_context(tc.tile_pool(name="headp", bufs=1))

        obs_top = headp.tile([128, GPC * NJ], F32, tag="obs_top")
        hp_all = headp.tile([128, GPC], F32, tag="hp_all")

        p1sb, y1sb, h1nm_g, x2sb, y2sb = {}, {}, {}, {}, {}

        def mm_tile():
            return psum.tile([128, 1000], F32, tag="mm", name="mm", bufs=3)

        def tp_tile():
            return psum.tile([128, NT * 128], F16, tag="tp", name="tp", bufs=2)

        adj_res = {}

        def load_adj_slabs(g, keep=True):
            """Strided DMAs of [128, SLAB, 1000] covering the graph's adjT."""
            slabs = []
            for s in range(NT // SLAB):
                sl = adj_res_p.tile([128, SLAB, NN], F16, tag="adjr", name="adjr")
                src = adjT[g, s * SLAB * 128:(s + 1) * SLAB * 128, :]
                nc.sync.dma_start(out=sl[:], in_=src.rearrange("(u p) n -> p u n", p=128))
                slabs.append(sl)
            adj_res[g] = slabs
            return slabs

        def adj_rhs(slabs, t, c0, cw):
            return slabs[t // SLAB][:, t % SLAB, c0:c0 + cw]

        def bn_sync(idx, gamma_col, beta_col):
            """AllReduce per-feature stats -> per-partition affine (a, c)."""
            mv = small.tile([H, 2], F32, tag=f"mv{idx}", name=f"mv{idx}")
            nc.vector.bn_aggr(out=mv[:], in_=stats_t[idx][:])
            ssum = small.tile([H, 2], F32, tag=f"ssum{idx}", name=f"ssum{idx}")
            msq = small.tile([H, 1], F32, tag=f"msq{idx}", name=f"msq{idx}")
            nc.vector.tensor_mul(out=msq[:], in0=mv[:, 0:1], in1=mv[:, 0:1])
            nc.vector.tensor_add(out=ssum[:, 1:2], in0=mv[:, 1:2], in1=msq[:])
            nc.vector.tensor_scalar_mul(out=ssum[:, 1:2], in0=ssum[:, 1:2], scalar1=ROWS_CORE)
            nc.vector.tensor_scalar_mul(out=ssum[:, 0:1], in0=mv[:, 0:1], scalar1=ROWS_CORE)
            cc_in = dram.tile([H, 2], F32, tag=f"ccin{idx}", name=f"ccin{idx}")
            cc_out = dram.tile([N_CORES, H, 2], F32, tag=f"ccout{idx}", name=f"ccout{idx}",
                               addr_space="Shared")
            # gpsimd queue: keeps the tiny sync DMA out of the bulk-DMA SP stream
            nc.gpsimd.dma_start(out=cc_in[:], in_=ssum[:])
            # AllGather has ~2x lower latency than AllReduce at this size; the
            # [128,2] payload is byte-reinterpreted as [16,16] so the gathered
            # output stays within 128 partitions, then reduced locally.
            nc.gpsimd.collective_compute(
                "AllGather", ALU.bypass, replica_groups=RG,
                ins=[cc_in[:].flatten().rearrange("(p c) -> p c", c=16).opt()],
                outs=[cc_out[:].flatten().rearrange("(p c) -> p c", c=16).opt()],
            )
            gl8 = small.tile([H, N_CORES, 2], F32, tag=f"gl8{idx}", name=f"gl8{idx}")
            nc.gpsimd.dma_start(out=gl8[:], in_=cc_out[:].rearrange("r f s -> f r s"))
            gl = small.tile([H, 2], F32, tag=f"gl{idx}", name=f"gl{idx}")
            nc.vector.reduce_sum(out=gl[:], in_=gl8[:].rearrange("f r s -> f s r"),
                                 axis=mybir.AxisListType.X)
            mu = small.tile([H, 1], F32, tag=f"mu{idx}", name=f"mu{idx}")
            var = small.tile([H, 1], F32, tag=f"var{idx}", name=f"var{idx}")
            nc.vector.tensor_scalar_mul(out=mu[:], in0=gl[:, 0:1], scalar1=1.0 / ROWS_TOT)
            nc.vector.tensor_scalar_mul(out=var[:], in0=gl[:, 1:2], scalar1=1.0 / ROWS_TOT)
            nc.vector.tensor_mul(out=msq[:], in0=mu[:], in1=mu[:])
            nc.vector.tensor_sub(out=var[:], in0=var[:], in1=msq[:])
            std = small.tile([H, 1], F32, tag=f"std{idx}", name=f"std{idx}")
            nc.scalar.activation(out=std[:], in_=var[:], func=AF.Sqrt, bias=eps_t[:], scale=1.0)
            rstd = small.tile([H, 1], F32, tag=f"rstd{idx}", name=f"rstd{idx}")
            nc.vector.reciprocal(out=rstd[:], in_=std[:])
            a = small.tile([H, 1], F32, tag=f"a{idx}", name=f"a{idx}")
            c = small.tile([H, 1], F32, tag=f"c{idx}", name=f"c{idx}")
            nc.vector.tensor_mul(out=a[:], in0=rstd[:], in1=gamma_col)
            nc.vector.tensor_mul(out=c[:], in0=a[:], in1=mu[:])
            nc.vector.tensor_sub(out=c[:], in0=beta_col, in1=c[:])
            return a, c

        # ================= Phase A: pool1 + x1 stats =================
        adj_ctx = ExitStack()
        adj_res_p = adj_ctx.enter_context(tc.tile_pool(name="adjres", bufs=RES * (NT // SLAB)))
        work_p = adj_ctx.enter_context(tc.tile_pool(name="workp", bufs=2))
        p1half = []
        for hh in range(2):
            p1h = p1sb_p.tile([2, GPC // 2 * NN], F16, tag="p1sb", name="p1sb")
            nc.sync.dma_start(out=p1h[:], in_=p1t[hh])
            p1half.append(p1h)
        for g in range(GPC):
            p1sb[g] = p1half[g // 4][:, (g % 4) * NN:(g % 4 + 1) * NN]
            xps = mm_tile()
            for (c0, cw) in CH:
                nc.tensor.matmul(xps[:, c0:c0 + cw], w1_sb[:], p1sb[g][:, c0:c0 + cw],
                                 start=True, stop=True)
            for ci, (c0, cw) in enumerate(CH):
                nc.vector.bn_stats(out=stats_t[0][:, 2 * g + ci, :], in_=xps[:, c0:c0 + cw])

        for g in range(GPC):
            load_adj_slabs(g)  # layer-2 prefetch; overlaps AG1/B/AG2 in the DMA engines

        a1, c1 = bn_sync(0, vecs1_sb[:, 0:1], vecs1_sb[:, 1:2])

        # ================= Phase B: z1 -> y1 + stats =================
        for g in range(GPC):
            xps = mm_tile()
            for (c0, cw) in CH:
                nc.tensor.matmul(xps[:, c0:c0 + cw], w1_sb[:], p1sb[g][:, c0:c0 + cw],
                                 start=True, stop=True)
            z1 = work_p.tile([128, NN], F16, tag="z1", name="z1")
            nc.scalar.activation(out=z1[:], in_=xps[:], func=AF.Relu, bias=c1[:], scale=a1[:])
            yps = mm_tile()
            for (c0, cw) in CH:
                nc.tensor.matmul(yps[:, c0:c0 + cw], w2_sb[:], z1[:, c0:c0 + cw],
                                 start=True, stop=True)
            for ci, (c0, cw) in enumerate(CH):
                nc.vector.bn_stats(out=stats_t[1][:, 2 * g + ci, :], in_=yps[:, c0:c0 + cw])
            y1 = y1sb_p.tile([128, NN], F16, tag="y1sb", name="y1sb")
            y1sb[g] = y1
            nc.scalar.activation(out=y1[:], in_=yps[:], func=AF.Copy)

        a2, c2 = bn_sync(1, vecs1_sb[:, 2:3], vecs1_sb[:, 3:4])

        # ===== Phase C: h1 = relu(BN(y1)), transpose, pool2, x2 stats =====
        for g in range(GPC):
            h1t = trans_p.tile([128, NPAD], F16, tag="h1t", name="h1t")
            nc.scalar.activation(out=h1t[:, 0:NN], in_=y1sb[g][:], func=AF.Relu,
                                 bias=c2[:], scale=a2[:])
            nc.vector.memset(h1t[:, NN:NPAD], 0.0)
            h1nm = h1nm_p.tile([128, NT * 128], F16, tag="h1nm", name="h1nm")
            h1nm_g[g] = h1nm
            tp = tp_tile()
            for t in range(NT):
                nc.tensor.transpose(tp[:, t * 128:(t + 1) * 128],
                                    h1t[:, t * 128:(t + 1) * 128], ident16[:])
            half = NT * 64
            nc.vector.tensor_copy(out=h1nm[:, 0:half], in_=tp[:, 0:half])
            nc.scalar.activation(out=h1nm[:, half:], in_=tp[:, half:], func=AF.Copy)
            slabs = adj_res[g]
            p2ps = mm_tile()
            for t in range(NT):
                for (c0, cw) in CH:
                    nc.tensor.matmul(p2ps[:, c0:c0 + cw], h1nm[:, t * 128:(t + 1) * 128],
                                     adj_rhs(slabs, t, c0, cw), start=(t == 0), stop=(t == NT - 1))
            p2 = work_p.tile([128, NN], F16, tag="p2sb", name="p2sb")
            nc.scalar.activation(out=p2[:], in_=p2ps[:], func=AF.Copy)
            x2ps = mm_tile()
            for (c0, cw) in CH:
                nc.tensor.matmul(x2ps[:, c0:c0 + cw], w1p_sb[:], p2[:, c0:c0 + cw],
                                 start=True, stop=True)
            for ci, (c0, cw) in enumerate(CH):
                nc.vector.bn_stats(out=stats_t[2][:, 2 * g + ci, :], in_=x2ps[:, c0:c0 + cw])
            x2 = sync2_p.tile([128, NN], F16, tag="s2", name="x2sb")
            x2sb[g] = x2
            nc.vector.tensor_copy(out=x2[:], in_=x2ps[:])

        adj_ctx.close()
        late_p = ctx.enter_context(tc.tile_pool(name="latep", bufs=2))
        gm_p = ctx.enter_context(tc.tile_pool(name="gmp", bufs=3))

        a3, c3 = bn_sync(2, vecs2_sb[:, 0:1], vecs2_sb[:, 1:2])

        # ================= Phase D: z2 -> y2 + stats =================
        for g in range(GPC):
            z2 = late_p.tile([128, NN], F16, tag="z2", name="z2")
            nc.scalar.activation(out=z2[:], in_=x2sb[g][:], func=AF.Relu, bias=c3[:], scale=a3[:])
            yps = mm_tile()
            for (c0, cw) in CH:
                nc.tensor.matmul(yps[:, c0:c0 + cw], w2p_sb[:], z2[:, c0:c0 + cw],
                                 start=True, stop=True)
            for ci, (c0, cw) in enumerate(CH):
                nc.vector.bn_stats(out=stats_t[3][:, 2 * g + ci, :], in_=yps[:, c0:c0 + cw])
            y2 = sync2_p.tile([128, NN], F16, tag="s2", name="y2sb")
            y2sb[g] = y2
            nc.scalar.activation(out=y2[:], in_=yps[:], func=AF.Copy)

        a4, c4 = bn_sync(3, vecs2_sb[:, 2:3], vecs2_sb[:, 3:4])

        # ===== Phase E: h2, transpose, gather (candidates + graph pool) =====
        for g in range(GPC):
            h2t = trans_p.tile([128, NPAD], F16, tag="h2t", name="h2t")
            nc.scalar.activation(out=h2t[:, 0:NN], in_=y2sb[g][:], func=AF.Relu,
                                 bias=c4[:], scale=a4[:])
            nc.vector.memset(h2t[:, NN:NPAD], 0.0)
            h2nm = late_p.tile([128, NT * 128], F16, tag="h2nm", name="h2nm")
            tp = tp_tile()
            for t in range(NT):
                nc.tensor.transpose(tp[:, t * 128:(t + 1) * 128],
                                    h2t[:, t * 128:(t + 1) * 128], ident16[:])
            half = NT * 64
            nc.vector.tensor_copy(out=h2nm[:, 0:half], in_=tp[:, 0:half])
            nc.scalar.activation(out=h2nm[:, half:], in_=tp[:, half:], func=AF.Copy)
            gm = gm_p.tile([128, NT, GCOLS], F16, tag="gm", name="gm")
            nc.sync.dma_start(out=gm[:], in_=gmat[g].rearrange("t p c -> p t c"))
            gps = mm_tile()
            for t in range(NT):
                nc.tensor.matmul(gps[:, 0:GCOLS], h2nm[:, t * 128:(t + 1) * 128], gm[:, t, :],
                                 start=(t == 0), stop=(t == NT - 1))
            nc.vector.tensor_copy(out=obs_top[:, g * NJ:(g + 1) * NJ], in_=gps[:, 0:NJ])
            nc.vector.tensor_copy(out=hp_all[:, g:g + 1], in_=gps[:, NJ:NJ + 1])

        # ================= Phase F: heads + dueling =================
        with tc.tile_pool(name="hf", bufs=1) as hf:
            def _loadh(ap, shape, tag):
                t = hf.tile(shape, F32, tag=tag, name=tag)
                nc.sync.dma_start(out=t[:], in_=ap)
                return t

            wc1_t, wc2_t, wv2_t = {}, {}, {}
            for kh in range(2):
                for oh in range(2):
                    wc1_t[(kh, oh)] = _loadh(wc1[kh * 128:(kh + 1) * 128, oh * 128:(oh + 1) * 128],
                                             [128, 128], f"wc1_{kh}{oh}")
                    wc2_t[(kh, oh)] = _loadh(wc2[kh * 128:(kh + 1) * 128, oh * 128:(oh + 1) * 128],
                                             [128, 128], f"wc2_{kh}{oh}")
                    wv2_t[(kh, oh)] = _loadh(wv2[kh * 128:(kh + 1) * 128, oh * 128:(oh + 1) * 128],
                                             [128, 128], f"wv2_{kh}{oh}")
            wv1_t = [_loadh(wv1[:, oh * 128:(oh + 1) * 128], [128, 128], f"wv1_{oh}")
                     for oh in range(2)]
            wc3_h = [_loadh(wc3[kh * 128:(kh + 1) * 128, :], [128, 1], f"wc3_{kh}")
                     for kh in range(2)]
            wv3_h = [_loadh(wv3[kh * 128:(kh + 1) * 128, :], [128, 1], f"wv3_{kh}")
                     for kh in range(2)]
            # critic bias path: vbb[oh] = W1c_bot.T @ h_pooled + b1c
            vbb = []
            for oh in range(2):
                ps = mm_tile()
                nc.tensor.matmul(ps[0:128, 0:GPC], wc1_t[(1, oh)][:], hp_all[:], start=True, stop=True)
                vb = hf.tile([128, GPC], F32, tag=f"vbb{oh}", name=f"vbb{oh}")
                nc.vector.tensor_scalar_add(out=vb[:], in0=ps[0:128, 0:GPC], scalar1=bc_sb[:, oh:oh + 1])
                vbb.append(vb)
            # critic L1
            a1c = []
            for oh in range(2):
                ps = mm_tile()
                nc.tensor.matmul(ps[:, 0:GPC * NJ], wc1_t[(0, oh)][:], obs_top[:], start=True, stop=True)
                acts = hf.tile([128, GPC * NJ], F32, tag=f"a1c{oh}", name=f"a1c{oh}")
                for g in range(GPC):
                    nc.scalar.activation(out=acts[:, g * NJ:(g + 1) * NJ],
                                         in_=ps[:, g * NJ:(g + 1) * NJ],
                                         func=AF.Relu, bias=vbb[oh][:, g:g + 1], scale=1.0)
                a1c.append(acts)
            # critic L2
            a2c = []
            for oh in range(2):
                ps = mm_tile()
                nc.tensor.matmul(ps[:, 0:GPC * NJ], wc2_t[(0, oh)][:], a1c[0][:], start=True, stop=False)
                nc.tensor.matmul(ps[:, 0:GPC * NJ], wc2_t[(1, oh)][:], a1c[1][:], start=False, stop=True)
                acts = hf.tile([128, GPC * NJ], F32, tag=f"a2c{oh}", name=f"a2c{oh}")
                nc.scalar.activation(out=acts[:], in_=ps[:, 0:GPC * NJ], func=AF.Relu,
                                     bias=bc_sb[:, 2 + oh:3 + oh], scale=1.0)
                a2c.append(acts)
            # critic L3 -> q_row [1, 400]
            psq3 = mm_tile()
            nc.tensor.matmul(psq3[0:1, 0:GPC * NJ], wc3_h[0][:], a2c[0][:], start=True, stop=False)
            nc.tensor.matmul(psq3[0:1, 0:GPC * NJ], wc3_h[1][:], a2c[1][:], start=False, stop=True)
            qrow = hf.tile([1, GPC * NJ], F32, tag="qrow", name="qrow")
            nc.vector.tensor_copy(out=qrow[:], in_=psq3[0:1, 0:GPC * NJ])
            # value head
            a1v = []
            for oh in range(2):
                ps = mm_tile()
                nc.tensor.matmul(ps[0:128, 0:GPC], wv1_t[oh][:], hp_all[:], start=True, stop=True)
                acts = hf.tile([128, GPC], F32, tag=f"a1v{oh}", name=f"a1v{oh}")
                nc.scalar.activation(out=acts[:], in_=ps[0:128, 0:GPC], func=AF.Relu,
                                     bias=bv_sb[:, oh:oh + 1], scale=1.0)
                a1v.append(acts)
            a2v = []
            for oh in range(2):
                ps = mm_tile()
                nc.tensor.matmul(ps[0:128, 0:GPC], wv2_t[(0, oh)][:], a1v[0][:], start=True, stop=False)
                nc.tensor.matmul(ps[0:128, 0:GPC], wv2_t[(1, oh)][:], a1v[1][:], start=False, stop=True)
                acts = hf.tile([128, GPC], F32, tag=f"a2v{oh}", name=f"a2v{oh}")
                nc.scalar.activation(out=acts[:], in_=ps[0:128, 0:GPC], func=AF.Relu,
                                     bias=bv_sb[:, 2 + oh:3 + oh], scale=1.0)
                a2v.append(acts)
            psvf = mm_tile()
            nc.tensor.matmul(psvf[0:1, 0:GPC], wv3_h[0][:], a2v[0][:], start=True, stop=False)
            nc.tensor.matmul(psvf[0:1, 0:GPC], wv3_h[1][:], a2v[1][:], start=False, stop=True)
            vrow = hf.tile([1, GPC], F32, tag="vrow", name="vrow")
            nc.vector.tensor_copy(out=vrow[:], in_=psvf[0:1, 0:GPC])
            # dueling combine: out = v + b3v + q[action] - sum(q*mask)/sum(mask)
            qm = hf.tile([1, GPC * NJ], F32, tag="qm", name="qm")
            nc.vector.tensor_mul(out=qm[:], in0=qrow[:], in1=mask_sb[:])
            sqm = hf.tile([1, GPC], F32, tag="sqm", name="sqm")
            nc.vector.reduce_sum(out=sqm[:].unsqueeze(2),
                                 in_=qm[:].rearrange("p (g j) -> p g j", g=GPC),
                                 axis=mybir.AxisListType.X)
            masked = hf.tile([1, GPC], F32, tag="masked", name="masked")
            nc.vector.tensor_mul(out=masked[:], in0=sqm[:], in1=minv_sb[:])
            qs = hf.tile([1, GPC * NJ], F32, tag="qs", name="qs")
            nc.vector.tensor_mul(out=qs[:], in0=qrow[:], in1=oh_sb[:])
            qsel = hf.tile([1, GPC], F32, tag="qsel", name="qsel")
            nc.vector.reduce_sum(out=qsel[:].unsqueeze(2),
                                 in_=qs[:].rearrange("p (g j) -> p g j", g=GPC),
                                 axis=mybir.AxisListType.X)
            qf = hf.tile([1, GPC], F32, tag="qf", name="qf")
            nc.vector.tensor_add(out=qf[:], in0=vrow[:], in1=qsel[:])
            nc.vector.tensor_sub(out=qf[:], in0=qf[:], in1=masked[:])
            nc.vector.tensor_scalar_add(out=qf[:], in0=qf[:], scalar1=scal_sb[0:1, 0:1])
            nc.sync.dma_start(out=qout[:], in_=qf[:])
            if debug_dump:
                nc.sync.dma_start(out=dbg["dbg_p1"][:], in_=p1sb[0][:])
                nc.sync.dma_start(out=dbg["dbg_a1"][:], in_=a1[:])
                nc.sync.dma_start(out=dbg["dbg_c1"][:], in_=c1[:])
                nc.sync.dma_start(out=dbg["dbg_y1"][:], in_=y1sb[0][:])
                nc.sync.dma_start(out=dbg["dbg_h1nm"][:], in_=h1nm_g[0][:])
                nc.sync.dma_start(out=dbg["dbg_x2"][:], in_=x2sb[0][:])
                nc.sync.dma_start(out=dbg["dbg_y2"][:], in_=y2sb[0][:])
                nc.sync.dma_start(out=dbg["dbg_obs"][:], in_=obs_top[:])
                nc.sync.dma_start(out=dbg["dbg_hp"][:], in_=hp_all[:])
                nc.sync.dma_start(out=dbg["dbg_qrow"][:], in_=qrow[:])
                nc.sync.dma_start(out=dbg["dbg_vrow"][:], in_=vrow[:])
                nc.sync.dma_start(out=dbg["dbg_sqm"][:], in_=sqm[:])
                nc.sync.dma_start(out=dbg["dbg_qsel"][:], in_=qsel[:])

    nc.compile()
    return nc


def _get_nc():
    if "nc" not in _CACHE:
        _CACHE["nc"] = _build()
    return _CACHE["nc"]


def _prep_inputs(adj, features, candidate, graph_pool, actions, action_masks,
                 gnn_params, critic_params, value_params):
    adj = np.asarray(adj, dtype=np.float32)
    features = np.asarray(features, dtype=np.float32)
    candidate = np.asarray(candidate).astype(np.int64)
    graph_pool = np.asarray(graph_pool, dtype=np.float32)
    actions = np.asarray(actions).astype(np.int64)
    mask = np.asarray(action_masks).astype(np.float32)

    adjTp = np.zeros((B, NPAD, NN), np.float16)
    adjTp[:, :NN, :] = adj.transpose(0, 2, 1)
    pooled1 = np.matmul(adj, features)               # [B, N, 2] fp32 (layer-1 message passing)
    p1tp = pooled1.transpose(0, 2, 1).astype(np.float16)         # [B, 2, N]
    # per-core packing: [2 halves, 2 features, 4*N] with graphs concatenated on the free axis
    p1pack = p1tp.reshape(N_CORES, 2, GPC // 2, 2, NN).transpose(0, 1, 3, 2, 4) \
                 .reshape(N_CORES, 2, 2, GPC // 2 * NN)
    gmatp = np.zeros((B, NPAD, GCOLS), np.float16)
    gmatp[np.arange(B)[:, None], candidate, np.arange(NJ)[None, :]] = 1.0
    gmatp[:, :NN, NJ] = graph_pool
    gmatp = gmatp.reshape(B, NT, 128, GCOLS)

    p1, p2 = gnn_params
    def f32(x):
        return np.ascontiguousarray(np.asarray(x, dtype=np.float32))
    def f16(x):
        return np.ascontiguousarray(np.asarray(x, dtype=np.float16))

    w1 = f16(p1["W1"]); w2 = f16(p1["W2"])
    w1p = f16(p2["W1"]); w2p = f16(p2["W2"])
    vecs1 = np.stack([f32(p1["bn1_g"]), f32(p1["bn1_b"]),
                      f32(p1["bn_g"]), f32(p1["bn_b"])], axis=1)
    vecs2 = np.stack([f32(p2["bn1_g"]), f32(p2["bn1_b"]),
                      f32(p2["bn_g"]), f32(p2["bn_b"])], axis=1)
    wc1 = f32(critic_params["W1"]); wc2 = f32(critic_params["W2"]); wc3 = f32(critic_params["W3"])
    b1c = f32(critic_params["b1"]); b2c = f32(critic_params["b2"])
    b3c = float(np.asarray(critic_params["b3"]).reshape(-1)[0])  # cancels in dueling
    bcm = np.stack([b1c[:128], b1c[128:], b2c[:128], b2c[128:]], axis=1)
    wv1 = f32(value_params["W1"]); wv2 = f32(value_params["W2"]); wv3 = f32(value_params["W3"])
    b1v = f32(value_params["b1"]); b2v = f32(value_params["b2"])
    b3v = float(np.asarray(value_params["b3"]).reshape(-1)[0])
    bvm = np.stack([b1v[:128], b1v[128:], b2v[:128], b2v[128:]], axis=1)

    onehot = np.zeros((B, NJ), np.float32)
    onehot[np.arange(B), actions[:, 0]] = 1.0
    minv = (1.0 / np.maximum(mask.sum(axis=1), 1e-9)).astype(np.float32)
    scal = np.array([[b3v, b3c, 0.0, 0.0]], np.float32)

    shared = {
        "w1": w1, "w2": w2, "w1p": w1p, "w2p": w2p,
        "vecs1": np.ascontiguousarray(vecs1, np.float32),
        "vecs2": np.ascontiguousarray(vecs2, np.float32),
        "wc1": wc1, "wc2": wc2, "wc3": wc3.reshape(HC, 1),
        "bc": np.ascontiguousarray(bcm, np.float32),
        "wv1": wv1, "wv2": wv2, "wv3": wv3.reshape(HC, 1),
        "bv": np.ascontiguousarray(bvm, np.float32),
        "scal": scal,
    }
    in_maps = []
    for c in range(N_CORES):
        s = slice(c * GPC, (c + 1) * GPC)
        m = dict(shared)
        m["adjT"] = np.ascontiguousarray(adjTp[s])
        m["p1t"] = np.ascontiguousarray(p1pack[c])
        m["gmat"] = np.ascontiguousarray(gmatp[s])
        m["maskr"] = np.ascontiguousarray(mask[s].reshape(1, GPC * NJ))
        m["ohr"] = np.ascontiguousarray(onehot[s].reshape(1, GPC * NJ))
        m["minv"] = np.ascontiguousarray(minv[s].reshape(1, GPC))
        in_maps.append(m)
    return in_maps


def kernel(adj, features, candidate, graph_pool, actions, action_masks,
           gnn_params, critic_params, value_params, _trace=False):
    nc = _get_nc()
    in_maps = _prep_inputs(adj, features, candidate, graph_pool, actions,
                           action_masks, gnn_params, critic_params, value_params)
    res = run_bass_kernel_spmd(nc, in_maps, list(range(N_CORES)), trace=_trace)
    out = np.concatenate([np.asarray(res.results[c]["qout"][0]) for c in range(N_CORES)])
    if _trace:
        _CACHE["last_results"] = res
    return out.astype(np.float32)


if __name__ == "__main__":
    nc = _get_nc()
    print("build + compile OK")


# revision 34
# speedup vs baseline: 1.0103x; 1.0103x over previous
"""Trainium2 Bass kernel for nn_DiscreteFullyConnectedQFunction (GIN message passing + dueling Q heads).

Strategy: data-parallel over batch (8 graphs per core, 8 cores). All heavy
matmuls in fp16 at full PE rate (T-layout, feature dim on partitions, adjT
streamed as the moving operand); the full 16MB fp16 adjT stays SBUF-resident
so it is read from HBM exactly once, prefetched under the early phases.
Layer-1 pooling (DIN=2, 1.6% of FLOPs) is done on host in fp32 so the adjT
load never gates the first BN sync. Exact training-mode BatchNorm via four
tiny cross-core AllGathers of per-feature (sum, sumsq) — the [128,2] payload
is byte-reinterpreted as [16,16] so the 8-rank concat fits 128 partitions,
then reduced locally (AG has ~2x lower latency than AllReduce). Candidate
gather and graph pooling are folded into one matmul against a host-built
one-hot/pool matrix. Linear biases inside the GIN blocks cancel exactly
under BatchNorm and are dropped. Heads run in fp32.
"""
import numpy as np
from contextlib import ExitStack

import concourse.bass as bass
import concourse.bacc as bacc
import concourse.tile as tile
import concourse.mybir as mybir
from concourse.bass_utils import run_bass_kernel_spmd
from concourse.masks import make_identity

N_CORES = 8
B, NN, NJ = 64, 1000, 50
GPC = B // N_CORES          # graphs per core
H, HC = 128, 256
NT = 8                      # node tiles of 128 (1000 padded to 1024)
NPAD = NT * 128
BN_EPS = 1e-5
ROWS_CORE = float(GPC * NN)  # BN rows per core
ROWS_TOT = float(B * NN)     # BN rows globally
GCOLS = 52                  # 50 candidate one-hot cols + graph_pool + pad
CH = [(0, 512), (512, 488)]  # free-dim chunks, aligned to 2KB PSUM banks
SLAB = 4                    # adjT m-tiles per DMA
RES = 8                     # graphs whose adjT stays SBUF-resident for pass 2
F16 = mybir.dt.float16
F32 = mybir.dt.float32
AF = mybir.ActivationFunctionType
ALU = mybir.AluOpType
RG = [list(range(N_CORES))]

_CACHE = {}


def _build(debug_dump=False):
    nc = bacc.Bacc("TRN2", target_bir_lowering=False, debug=False, num_devices=N_CORES)
    dbg = {}
    if debug_dump:
        for nm, shape, dt in [
            ("dbg_p1", [2, NN], F16), ("dbg_a1", [H, 1], F32), ("dbg_c1", [H, 1], F32),
            ("dbg_y1", [H, NN], F16), ("dbg_h1nm", [128, NT * 128], F16),
            ("dbg_x2", [H, NN], F16), ("dbg_y2", [H, NN], F16),
            ("dbg_obs", [128, GPC * NJ], F32), ("dbg_hp", [128, GPC], F32),
            ("dbg_qrow", [1, GPC * NJ], F32), ("dbg_vrow", [1, GPC], F32),
            ("dbg_sqm", [1, GPC], F32), ("dbg_qsel", [1, GPC], F32),
        ]:
            dbg[nm] = nc.declare_dram_parameter(nm, shape, dt, isOutput=True)

    adjT = nc.declare_dram_parameter("adjT", [GPC, NPAD, NN], F16, isOutput=False)
    p1t = nc.declare_dram_parameter("p1t", [2, 2, GPC // 2 * NN], F16, isOutput=False)
    gmat = nc.declare_dram_parameter("gmat", [GPC, NT, 128, GCOLS], F16, isOutput=False)
    w1 = nc.declare_dram_parameter("w1", [2, H], F16, isOutput=False)
    w2 = nc.declare_dram_parameter("w2", [H, H], F16, isOutput=False)
    w1p = nc.declare_dram_parameter("w1p", [H, H], F16, isOutput=False)
    w2p = nc.declare_dram_parameter("w2p", [H, H], F16, isOutput=False)
    vecs1 = nc.declare_dram_parameter("vecs1", [H, 4], F32, isOutput=False)  # g_in,b_in,g_out,b_out
    vecs2 = nc.declare_dram_parameter("vecs2", [H, 4], F32, isOutput=False)
    wc1 = nc.declare_dram_parameter("wc1", [2 * H, HC], F16, isOutput=False)
    wc2 = nc.declare_dram_parameter("wc2", [HC, HC], F16, isOutput=False)
    wc3 = nc.declare_dram_parameter("wc3", [HC, 1], F16, isOutput=False)
    bc = nc.declare_dram_parameter("bc", [H, 4], F32, isOutput=False)  # b1c lo/hi, b2c lo/hi
    wv1 = nc.declare_dram_parameter("wv1", [H, HC], F16, isOutput=False)
    wv2 = nc.declare_dram_parameter("wv2", [HC, HC], F16, isOutput=False)
    wv3 = nc.declare_dram_parameter("wv3", [HC, 1], F16, isOutput=False)
    bv = nc.declare_dram_parameter("bv", [H, 4], F32, isOutput=False)
    maskr = nc.declare_dram_parameter("maskr", [1, GPC * NJ], F32, isOutput=False)
    ohr = nc.declare_dram_parameter("ohr", [1, GPC * NJ], F32, isOutput=False)
    minv = nc.declare_dram_parameter("minv", [1, GPC], F32, isOutput=False)
    scal = nc.declare_dram_parameter("scal", [1, 4], F32, isOutput=False)  # [b3v, ...]
    qout = nc.declare_dram_parameter("qout", [1, GPC], F32, isOutput=True)

    with tile.TileContext(nc) as tc, ExitStack() as ctx:
        const = ctx.enter_context(tc.tile_pool(name="const", bufs=1))
        small = ctx.enter_context(tc.tile_pool(name="small", bufs=1))
        dram = ctx.enter_context(tc.tile_pool(name="dram", bufs=1, space="DRAM"))
        statsp = ctx.enter_context(tc.tile_pool(name="stats", bufs=1))
        # one psum pool for the whole kernel: "mm" [128,1000] f32 x3 (6 banks)
        # + "tp" [128,128] f16 x2 (2 banks) = 8 banks
        psum = ctx.enter_context(tc.tile_pool(name="psum", bufs=1, space="PSUM"))

        # ---- constants / params in SBUF
        ident16 = const.tile([128, 128], F16, tag="ident")
        make_identity(nc, ident16[:])
        eps_t = const.tile([128, 1], F32, tag="eps")
        nc.vector.memset(eps_t[:], BN_EPS)

        def _load(name, ap, shape, dt, tag):
            t = const.tile(shape, dt, tag=tag, name=tag)
            nc.sync.dma_start(out=t[:], in_=ap)
            return t

        w1_sb = _load("w1", w1[:], [2, H], F16, "w1")
        w2_sb = _load("w2", w2[:], [H, H], F16, "w2")
        w1p_sb = _load("w1p", w1p[:], [H, H], F16, "w1p")
        w2p_sb = _load("w2p", w2p[:], [H, H], F16, "w2p")
        vecs1_sb = _load("vecs1", vecs1[:], [H, 4], F32, "vecs1")
        vecs2_sb = _load("vecs2", vecs2[:], [H, 4], F32, "vecs2")
        bc_sb = _load("bc", bc[:], [H, 4], F32, "bc")
        bv_sb = _load("bv", bv[:], [H, 4], F32, "bv")
        mask_sb = _load("maskr", maskr[:], [1, GPC * NJ], F32, "maskr")
        oh_sb = _load("ohr", ohr[:], [1, GPC * NJ], F32, "ohr")
        minv_sb = _load("minv", minv[:], [1, GPC], F32, "minv")
        scal_sb = _load("scal", scal[:], [1, 4], F32, "scal")


        stats_t = [statsp.tile([H, 2 * GPC, 6], F32, tag=f"stats{i}", name=f"stats{i}")
                   for i in range(4)]

        # ---- persistent activation pools
        p1sb_p = ctx.enter_context(tc.tile_pool(name="p1sb", bufs=2))
        y1sb_p = ctx.enter_context(tc.tile_pool(name="y1sb", bufs=GPC))
        h1nm_p = ctx.enter_context(tc.tile_pool(name="h1nm", bufs=2))
        sync2_p = ctx.enter_context(tc.tile_pool(name="sync2", bufs=GPC + 1))
        trans_p = ctx.enter_context(tc.tile_pool(name="trans", bufs=2))
        headp = ctx.enter_context(tc.tile_pool(name="headp", bufs=1))

        obs_top = headp.tile([128, GPC * NJ], F16, tag="obs_top")
        hp_all = headp.tile([128, GPC], F16, tag="hp_all")

        p1sb, y1sb, h1nm_g, x2sb, y2sb = {}, {}, {}, {}, {}

        def mm_tile():
            return psum.tile([128, 1000], F32, tag="mm", name="mm", bufs=3)

        def tp_tile():
            return psum.tile([128, NT * 128], F16, tag="tp", name="tp", bufs=2)

        adj_res = {}

        def load_adj_slabs(g, keep=True):
            """Strided DMAs of [128, SLAB, 1000] covering the graph's adjT."""
            slabs = []
            for s in range(NT // SLAB):
                sl = adj_res_p.tile([128, SLAB, NN], F16, tag="adjr", name="adjr")
                src = adjT[g, s * SLAB * 128:(s + 1) * SLAB * 128, :]
                nc.sync.dma_start(out=sl[:], in_=src.rearrange("(u p) n -> p u n", p=128))
                slabs.append(sl)
            adj_res[g] = slabs
            return slabs

        def adj_rhs(slabs, t, c0, cw):
            return slabs[t // SLAB][:, t % SLAB, c0:c0 + cw]

        def bn_sync(idx, gamma_col, beta_col):
            """AllReduce per-feature stats -> per-partition affine (a, c)."""
            mv = small.tile([H, 2], F32, tag=f"mv{idx}", name=f"mv{idx}")
            nc.vector.bn_aggr(out=mv[:], in_=stats_t[idx][:])
            ssum = small.tile([H, 2], F32, tag=f"ssum{idx}", name=f"ssum{idx}")
            msq = small.tile([H, 1], F32, tag=f"msq{idx}", name=f"msq{idx}")
            nc.vector.tensor_mul(out=msq[:], in0=mv[:, 0:1], in1=mv[:, 0:1])
            nc.vector.tensor_add(out=ssum[:, 1:2], in0=mv[:, 1:2], in1=msq[:])
            nc.vector.tensor_scalar_mul(out=ssum[:, 1:2], in0=ssum[:, 1:2], scalar1=ROWS_CORE)
            nc.vector.tensor_scalar_mul(out=ssum[:, 0:1], in0=mv[:, 0:1], scalar1=ROWS_CORE)
            cc_in = dram.tile([H, 2], F32, tag=f"ccin{idx}", name=f"ccin{idx}")
            cc_out = dram.tile([N_CORES, H, 2], F32, tag=f"ccout{idx}", name=f"ccout{idx}",
                               addr_space="Shared")
            # gpsimd queue: keeps the tiny sync DMA out of the bulk-DMA SP stream
            nc.gpsimd.dma_start(out=cc_in[:], in_=ssum[:])
            # AllGather has ~2x lower latency than AllReduce at this size; the
            # [128,2] payload is byte-reinterpreted as [16,16] so the gathered
            # output stays within 128 partitions, then reduced locally.
            nc.gpsimd.collective_compute(
                "AllGather", ALU.bypass, replica_groups=RG,
                ins=[cc_in[:].flatten().rearrange("(p c) -> p c", c=16).opt()],
                outs=[cc_out[:].flatten().rearrange("(p c) -> p c", c=16).opt()],
            )
            gl8 = small.tile([H, N_CORES, 2], F32, tag=f"gl8{idx}", name=f"gl8{idx}")
            nc.gpsimd.dma_start(out=gl8[:], in_=cc_out[:].rearrange("r f s -> f r s"))
            gl = small.tile([H, 2], F32, tag=f"gl{idx}", name=f"gl{idx}")
            nc.vector.reduce_sum(out=gl[:], in_=gl8[:].rearrange("f r s -> f s r"),
                                 axis=mybir.AxisListType.X)
            mu = small.tile([H, 1], F32, tag=f"mu{idx}", name=f"mu{idx}")
            var = small.tile([H, 1], F32, tag=f"var{idx}", name=f"var{idx}")
            nc.vector.tensor_scalar_mul(out=mu[:], in0=gl[:, 0:1], scalar1=1.0 / ROWS_TOT)
            nc.vector.tensor_scalar_mul(out=var[:], in0=gl[:, 1:2], scalar1=1.0 / ROWS_TOT)
            nc.vector.tensor_mul(out=msq[:], in0=mu[:], in1=mu[:])
            nc.vector.tensor_sub(out=var[:], in0=var[:], in1=msq[:])
            std = small.tile([H, 1], F32, tag=f"std{idx}", name=f"std{idx}")
            nc.scalar.activation(out=std[:], in_=var[:], func=AF.Sqrt, bias=eps_t[:], scale=1.0)
            rstd = small.tile([H, 1], F32, tag=f"rstd{idx}", name=f"rstd{idx}")
            nc.vector.reciprocal(out=rstd[:], in_=std[:])
            a = small.tile([H, 1], F32, tag=f"a{idx}", name=f"a{idx}")
            c = small.tile([H, 1], F32, tag=f"c{idx}", name=f"c{idx}")
            nc.vector.tensor_mul(out=a[:], in0=rstd[:], in1=gamma_col)
            nc.vector.tensor_mul(out=c[:], in0=a[:], in1=mu[:])
            nc.vector.tensor_sub(out=c[:], in0=beta_col, in1=c[:])
            return a, c

        # ================= Phase A: pool1 + x1 stats =================
        adj_ctx = ExitStack()
        adj_res_p = adj_ctx.enter_context(tc.tile_pool(name="adjres", bufs=RES * (NT // SLAB)))
        work_p = adj_ctx.enter_context(tc.tile_pool(name="workp", bufs=2))
        p1half = []
        for hh in range(2):
            p1h = p1sb_p.tile([2, GPC // 2 * NN], F16, tag="p1sb", name="p1sb")
            nc.sync.dma_start(out=p1h[:], in_=p1t[hh])
            p1half.append(p1h)
        for g in range(GPC):
            p1sb[g] = p1half[g // 4][:, (g % 4) * NN:(g % 4 + 1) * NN]
            xps = mm_tile()
            for (c0, cw) in CH:
                nc.tensor.matmul(xps[:, c0:c0 + cw], w1_sb[:], p1sb[g][:, c0:c0 + cw],
                                 start=True, stop=True)
            for ci, (c0, cw) in enumerate(CH):
                nc.vector.bn_stats(out=stats_t[0][:, 2 * g + ci, :], in_=xps[:, c0:c0 + cw])

        for g in range(GPC):
            load_adj_slabs(g)  # layer-2 prefetch; overlaps AG1/B/AG2 in the DMA engines

        a1, c1 = bn_sync(0, vecs1_sb[:, 0:1], vecs1_sb[:, 1:2])

        # ================= Phase B: z1 -> y1 + stats =================
        for g in range(GPC):
            xps = mm_tile()
            for (c0, cw) in CH:
                nc.tensor.matmul(xps[:, c0:c0 + cw], w1_sb[:], p1sb[g][:, c0:c0 + cw],
                                 start=True, stop=True)
            z1 = work_p.tile([128, NN], F16, tag="z1", name="z1")
            nc.scalar.activation(out=z1[:], in_=xps[:], func=AF.Relu, bias=c1[:], scale=a1[:])
            yps = mm_tile()
            for (c0, cw) in CH:
                nc.tensor.matmul(yps[:, c0:c0 + cw], w2_sb[:], z1[:, c0:c0 + cw],
                                 start=True, stop=True)
            for ci, (c0, cw) in enumerate(CH):
                nc.vector.bn_stats(out=stats_t[1][:, 2 * g + ci, :], in_=yps[:, c0:c0 + cw])
            y1 = y1sb_p.tile([128, NN], F16, tag="y1sb", name="y1sb")
            y1sb[g] = y1
            nc.scalar.activation(out=y1[:], in_=yps[:], func=AF.Copy)

        a2, c2 = bn_sync(1, vecs1_sb[:, 2:3], vecs1_sb[:, 3:4])

        # ===== Phase C: h1 = relu(BN(y1)), transpose, pool2, x2 stats =====
        for g in range(GPC):
            h1t = trans_p.tile([128, NPAD], F16, tag="h1t", name="h1t")
            nc.scalar.activation(out=h1t[:, 0:NN], in_=y1sb[g][:], func=AF.Relu,
                                 bias=c2[:], scale=a2[:])
            nc.vector.memset(h1t[:, NN:NPAD], 0.0)
            h1nm = h1nm_p.tile([128, NT * 128], F16, tag="h1nm", name="h1nm")
            h1nm_g[g] = h1nm
            tp = tp_tile()
            for t in range(NT):
                nc.tensor.transpose(tp[:, t * 128:(t + 1) * 128],
                                    h1t[:, t * 128:(t + 1) * 128], ident16[:])
            half = NT * 64
            nc.vector.tensor_copy(out=h1nm[:, 0:half], in_=tp[:, 0:half])
            nc.scalar.activation(out=h1nm[:, half:], in_=tp[:, half:], func=AF.Copy)
            slabs = adj_res[g]
            p2ps = mm_tile()
            for t in range(NT):
                for (c0, cw) in CH:
                    nc.tensor.matmul(p2ps[:, c0:c0 + cw], h1nm[:, t * 128:(t + 1) * 128],
                                     adj_rhs(slabs, t, c0, cw), start=(t == 0), stop=(t == NT - 1))
            p2 = work_p.tile([128, NN], F16, tag="p2sb", name="p2sb")
            nc.scalar.activation(out=p2[:], in_=p2ps[:], func=AF.Copy)
            x2ps = mm_tile()
            for (c0, cw) in CH:
                nc.tensor.matmul(x2ps[:, c0:c0 + cw], w1p_sb[:], p2[:, c0:c0 + cw],
                                 start=True, stop=True)
            for ci, (c0, cw) in enumerate(CH):
                nc.vector.bn_stats(out=stats_t[2][:, 2 * g + ci, :], in_=x2ps[:, c0:c0 + cw])
            x2 = sync2_p.tile([128, NN], F16, tag="s2", name="x2sb")
            x2sb[g] = x2
            nc.vector.tensor_copy(out=x2[:], in_=x2ps[:])

        adj_ctx.close()
        late_p = ctx.enter_context(tc.tile_pool(name="latep", bufs=2))
        gm_p = ctx.enter_context(tc.tile_pool(name="gmp", bufs=3))

        a3, c3 = bn_sync(2, vecs2_sb[:, 0:1], vecs2_sb[:, 1:2])

        # ================= Phase D: z2 -> y2 + stats =================
        for g in range(GPC):
            z2 = late_p.tile([128, NN], F16, tag="z2", name="z2")
            nc.scalar.activation(out=z2[:], in_=x2sb[g][:], func=AF.Relu, bias=c3[:], scale=a3[:])
            yps = mm_tile()
            for (c0, cw) in CH:
                nc.tensor.matmul(yps[:, c0:c0 + cw], w2p_sb[:], z2[:, c0:c0 + cw],
                                 start=True, stop=True)
            for ci, (c0, cw) in enumerate(CH):
                nc.vector.bn_stats(out=stats_t[3][:, 2 * g + ci, :], in_=yps[:, c0:c0 + cw])
            y2 = sync2_p.tile([128, NN], F16, tag="s2", name="y2sb")
            y2sb[g] = y2
            nc.scalar.activation(out=y2[:], in_=yps[:], func=AF.Copy)

        a4, c4 = bn_sync(3, vecs2_sb[:, 2:3], vecs2_sb[:, 3:4])

        # ===== Phase E: h2, transpose, gather (candidates + graph pool) =====
        for g in range(GPC):
            h2t = trans_p.tile([128, NPAD], F16, tag="h2t", name="h2t")
            nc.scalar.activation(out=h2t[:, 0:NN], in_=y2sb[g][:], func=AF.Relu,
                                 bias=c4[:], scale=a4[:])
            nc.vector.memset(h2t[:, NN:NPAD], 0.0)
            h2nm = late_p.tile([128, NT * 128], F16, tag="h2nm", name="h2nm")
            tp = tp_tile()
            for t in range(NT):
                nc.tensor.transpose(tp[:, t * 128:(t + 1) * 128],
                                    h2t[:, t * 128:(t + 1) * 128], ident16[:])
            half = NT * 64
            nc.vector.tensor_copy(out=h2nm[:, 0:half], in_=tp[:, 0:half])
            nc.scalar.activation(out=h2nm[:, half:], in_=tp[:, half:], func=AF.Copy)
            gm = gm_p.tile([128, NT, GCOLS], F16, tag="gm", name="gm")
            nc.sync.dma_start(out=gm[:], in_=gmat[g].rearrange("t p c -> p t c"))
            gps = mm_tile()
            for t in range(NT):
                nc.tensor.matmul(gps[:, 0:GCOLS], h2nm[:, t * 128:(t + 1) * 128], gm[:, t, :],
                                 start=(t == 0), stop=(t == NT - 1))
            nc.vector.tensor_copy(out=obs_top[:, g * NJ:(g + 1) * NJ], in_=gps[:, 0:NJ])
            nc.vector.tensor_copy(out=hp_all[:, g:g + 1], in_=gps[:, NJ:NJ + 1])

        # ================= Phase F: heads + dueling =================
        with tc.tile_pool(name="hf", bufs=1) as hf:
            def _loadh(ap, shape, tag):
                t = hf.tile(shape, F16, tag=tag, name=tag)
                nc.sync.dma_start(out=t[:], in_=ap)
                return t

            wc1_t, wc2_t, wv2_t = {}, {}, {}
            for kh in range(2):
                for oh in range(2):
                    wc1_t[(kh, oh)] = _loadh(wc1[kh * 128:(kh + 1) * 128, oh * 128:(oh + 1) * 128],
                                             [128, 128], f"wc1_{kh}{oh}")
                    wc2_t[(kh, oh)] = _loadh(wc2[kh * 128:(kh + 1) * 128, oh * 128:(oh + 1) * 128],
                                             [128, 128], f"wc2_{kh}{oh}")
                    wv2_t[(kh, oh)] = _loadh(wv2[kh * 128:(kh + 1) * 128, oh * 128:(oh + 1) * 128],
                                             [128, 128], f"wv2_{kh}{oh}")
            wv1_t = [_loadh(wv1[:, oh * 128:(oh + 1) * 128], [128, 128], f"wv1_{oh}")
                     for oh in range(2)]
            wc3_h = [_loadh(wc3[kh * 128:(kh + 1) * 128, :], [128, 1], f"wc3_{kh}")
                     for kh in range(2)]
            wv3_h = [_loadh(wv3[kh * 128:(kh + 1) * 128, :], [128, 1], f"wv3_{kh}")
                     for kh in range(2)]
            # critic bias path: vbb[oh] = W1c_bot.T @ h_pooled + b1c
            vbb = []
            for oh in range(2):
                ps = mm_tile()
                nc.tensor.matmul(ps[0:128, 0:GPC], wc1_t[(1, oh)][:], hp_all[:], start=True, stop=True)
                vb = hf.tile([128, GPC], F32, tag=f"vbb{oh}", name=f"vbb{oh}")
                nc.vector.tensor_scalar_add(out=vb[:], in0=ps[0:128, 0:GPC], scalar1=bc_sb[:, oh:oh + 1])
                vbb.append(vb)
            # critic L1
            a1c = []
            for oh in range(2):
                ps = mm_tile()
                nc.tensor.matmul(ps[:, 0:GPC * NJ], wc1_t[(0, oh)][:], obs_top[:], start=True, stop=True)
                acts = hf.tile([128, GPC * NJ], F16, tag=f"a1c{oh}", name=f"a1c{oh}")
                for g in range(GPC):
                    nc.scalar.activation(out=acts[:, g * NJ:(g + 1) * NJ],
                                         in_=ps[:, g * NJ:(g + 1) * NJ],
                                         func=AF.Relu, bias=vbb[oh][:, g:g + 1], scale=1.0)
                a1c.append(acts)
            # critic L2
            a2c = []
            for oh in range(2):
                ps = mm_tile()
                nc.tensor.matmul(ps[:, 0:GPC * NJ], wc2_t[(0, oh)][:], a1c[0][:], start=True, stop=False)
                nc.tensor.matmul(ps[:, 0:GPC * NJ], wc2_t[(1, oh)][:], a1c[1][:], start=False, stop=True)
                acts = hf.tile([128, GPC * NJ], F16, tag=f"a2c{oh}", name=f"a2c{oh}")
                nc.scalar.activation(out=acts[:], in_=ps[:, 0:GPC * NJ], func=AF.Relu,
                                     bias=bc_sb[:, 2 + oh:3 + oh], scale=1.0)
                a2c.append(acts)
            # critic L3 -> q_row [1, 400]
            psq3 = mm_tile()
            nc.tensor.matmul(psq3[0:1, 0:GPC * NJ], wc3_h[0][:], a2c[0][:], start=True, stop=False)
            nc.tensor.matmul(psq3[0:1, 0:GPC * NJ], wc3_h[1][:], a2c[1][:], start=False, stop=True)
            qrow = hf.tile([1, GPC * NJ], F32, tag="qrow", name="qrow")
            nc.vector.tensor_copy(out=qrow[:], in_=psq3[0:1, 0:GPC * NJ])
            # value head
            a1v = []
            for oh in range(2):
                ps = mm_tile()
                nc.tensor.matmul(ps[0:128, 0:GPC], wv1_t[oh][:], hp_all[:], start=True, stop=True)
                acts = hf.tile([128, GPC], F16, tag=f"a1v{oh}", name=f"a1v{oh}")
                nc.scalar.activation(out=acts[:], in_=ps[0:128, 0:GPC], func=AF.Relu,
                                     bias=bv_sb[:, oh:oh + 1], scale=1.0)
                a1v.append(acts)
            a2v = []
            for oh in range(2):
                ps = mm_tile()
                nc.tensor.matmul(ps[0:128, 0:GPC], wv2_t[(0, oh)][:], a1v[0][:], start=True, stop=False)
                nc.tensor.matmul(ps[0:128, 0:GPC], wv2_t[(1, oh)][:], a1v[1][:], start=False, stop=True)
                acts = hf.tile([128, GPC], F16, tag=f"a2v{oh}", name=f"a2v{oh}")
                nc.scalar.activation(out=acts[:], in_=ps[0:128, 0:GPC], func=AF.Relu,
                                     bias=bv_sb[:, 2 + oh:3 + oh], scale=1.0)
                a2v.append(acts)
            psvf = mm_tile()
            nc.tensor.matmul(psvf[0:1, 0:GPC], wv3_h[0][:], a2v[0][:], start=True, stop=False)
            nc.tensor.matmul(psvf[0:1, 0:GPC], wv3_h[1][:], a2v[1][:], start=False, stop=True)
            vrow = hf.tile([1, GPC], F32, tag="vrow", name="vrow")
            nc.vector.tensor_copy(out=vrow[:], in_=psvf[0:1, 0:GPC])
            # dueling combine: out = v + b3v + q[action] - sum(q*mask)/sum(mask)
            qm = hf.tile([1, GPC * NJ], F32, tag="qm", name="qm")
            nc.vector.tensor_mul(out=qm[:], in0=qrow[:], in1=mask_sb[:])
            sqm = hf.tile([1, GPC], F32, tag="sqm", name="sqm")
            nc.vector.reduce_sum(out=sqm[:].unsqueeze(2),
                                 in_=qm[:].rearrange("p (g j) -> p g j", g=GPC),
                                 axis=mybir.AxisListType.X)
            masked = hf.tile([1, GPC], F32, tag="masked", name="masked")
            nc.vector.tensor_mul(out=masked[:], in0=sqm[:], in1=minv_sb[:])
            qs = hf.tile([1, GPC * NJ], F32, tag="qs", name="qs")
            nc.vector.tensor_mul(out=qs[:], in0=qrow[:], in1=oh_sb[:])
            qsel = hf.tile([1, GPC], F32, tag="qsel", name="qsel")
            nc.vector.reduce_sum(out=qsel[:].unsqueeze(2),
                                 in_=qs[:].rearrange("p (g j) -> p g j", g=GPC),
                                 axis=mybir.AxisListType.X)
            qf = hf.tile([1, GPC], F32, tag="qf", name="qf")
            nc.vector.tensor_add(out=qf[:], in0=vrow[:], in1=qsel[:])
            nc.vector.tensor_sub(out=qf[:], in0=qf[:], in1=masked[:])
            nc.vector.tensor_scalar_add(out=qf[:], in0=qf[:], scalar1=scal_sb[0:1, 0:1])
            nc.sync.dma_start(out=qout[:], in_=qf[:])
            if debug_dump:
                nc.sync.dma_start(out=dbg["dbg_p1"][:], in_=p1sb[0][:])
                nc.sync.dma_start(out=dbg["dbg_a1"][:], in_=a1[:])
                nc.sync.dma_start(out=dbg["dbg_c1"][:], in_=c1[:])
                nc.sync.dma_start(out=dbg["dbg_y1"][:], in_=y1sb[0][:])
                nc.sync.dma_start(out=dbg["dbg_h1nm"][:], in_=h1nm_g[0][:])
                nc.sync.dma_start(out=dbg["dbg_x2"][:], in_=x2sb[0][:])
                nc.sync.dma_start(out=dbg["dbg_y2"][:], in_=y2sb[0][:])
                nc.sync.dma_start(out=dbg["dbg_obs"][:], in_=obs_top[:])
                nc.sync.dma_start(out=dbg["dbg_hp"][:], in_=hp_all[:])
                nc.sync.dma_start(out=dbg["dbg_qrow"][:], in_=qrow[:])
                nc.sync.dma_start(out=dbg["dbg_vrow"][:], in_=vrow[:])
                nc.sync.dma_start(out=dbg["dbg_sqm"][:], in_=sqm[:])
                nc.sync.dma_start(out=dbg["dbg_qsel"][:], in_=qsel[:])

    nc.compile()
    return nc


def _get_nc():
    if "nc" not in _CACHE:
        _CACHE["nc"] = _build()
    return _CACHE["nc"]


def _prep_inputs(adj, features, candidate, graph_pool, actions, action_masks,
                 gnn_params, critic_params, value_params):
    adj = np.asarray(adj, dtype=np.float32)
    features = np.asarray(features, dtype=np.float32)
    candidate = np.asarray(candidate).astype(np.int64)
    graph_pool = np.asarray(graph_pool, dtype=np.float32)
    actions = np.asarray(actions).astype(np.int64)
    mask = np.asarray(action_masks).astype(np.float32)

    adjTp = np.zeros((B, NPAD, NN), np.float16)
    adjTp[:, :NN, :] = adj.transpose(0, 2, 1)
    pooled1 = np.matmul(adj, features)               # [B, N, 2] fp32 (layer-1 message passing)
    p1tp = pooled1.transpose(0, 2, 1).astype(np.float16)         # [B, 2, N]
    # per-core packing: [2 halves, 2 features, 4*N] with graphs concatenated on the free axis
    p1pack = p1tp.reshape(N_CORES, 2, GPC // 2, 2, NN).transpose(0, 1, 3, 2, 4) \
                 .reshape(N_CORES, 2, 2, GPC // 2 * NN)
    gmatp = np.zeros((B, NPAD, GCOLS), np.float16)
    gmatp[np.arange(B)[:, None], candidate, np.arange(NJ)[None, :]] = 1.0
    gmatp[:, :NN, NJ] = graph_pool
    gmatp = gmatp.reshape(B, NT, 128, GCOLS)

    p1, p2 = gnn_params
    def f32(x):
        return np.ascontiguousarray(np.asarray(x, dtype=np.float32))
    def f16(x):
        return np.ascontiguousarray(np.asarray(x, dtype=np.float16))

    w1 = f16(p1["W1"]); w2 = f16(p1["W2"])
    w1p = f16(p2["W1"]); w2p = f16(p2["W2"])
    vecs1 = np.stack([f32(p1["bn1_g"]), f32(p1["bn1_b"]),
                      f32(p1["bn_g"]), f32(p1["bn_b"])], axis=1)
    vecs2 = np.stack([f32(p2["bn1_g"]), f32(p2["bn1_b"]),
                      f32(p2["bn_g"]), f32(p2["bn_b"])], axis=1)
    wc1 = f16(critic_params["W1"]); wc2 = f16(critic_params["W2"]); wc3 = f16(critic_params["W3"])
    b1c = f32(critic_params["b1"]); b2c = f32(critic_params["b2"])
    b3c = float(np.asarray(critic_params["b3"]).reshape(-1)[0])  # cancels in dueling
    bcm = np.stack([b1c[:128], b1c[128:], b2c[:128], b2c[128:]], axis=1)
    wv1 = f16(value_params["W1"]); wv2 = f16(value_params["W2"]); wv3 = f16(value_params["W3"])
    b1v = f32(value_params["b1"]); b2v = f32(value_params["b2"])
    b3v = float(np.asarray(value_params["b3"]).reshape(-1)[0])
    bvm = np.stack([b1v[:128], b1v[128:], b2v[:128], b2v[128:]], axis=1)

    onehot = np.zeros((B, NJ), np.float32)
    onehot[np.arange(B), actions[:, 0]] = 1.0
    minv = (1.0 / np.maximum(mask.sum(axis=1), 1e-9)).astype(np.float32)
    scal = np.array([[b3v, b3c, 0.0, 0.0]], np.float32)

    shared = {
        "w1": w1, "w2": w2, "w1p": w1p, "w2p": w2p,
        "vecs1": np.ascontiguousarray(vecs1, np.float32),
        "vecs2": np.ascontiguousarray(vecs2, np.float32),
        "wc1": wc1, "wc2": wc2, "wc3": wc3.reshape(HC, 1),
        "bc": np.ascontiguousarray(bcm, np.float32),
        "wv1": wv1, "wv2": wv2, "wv3": wv3.reshape(HC, 1),
        "bv": np.ascontiguousarray(bvm, np.float32),
        "scal": scal,
    }
    in_maps = []
    for c in range(N_CORES):
        s = slice(c * GPC, (c + 1) * GPC)
        m = dict(shared)
        m["adjT"] = np.ascontiguousarray(adjTp[s])
        m["p1t"] = np.ascontiguousarray(p1pack[c])
        m["gmat"] = np.ascontiguousarray(gmatp[s])
        m["maskr"] = np.ascontiguousarray(mask[s].reshape(1, GPC * NJ))
        m["ohr"] = np.ascontiguousarray(onehot[s].reshape(1, GPC * NJ))
        m["minv"] = np.ascontiguousarray(minv[s].reshape(1, GPC))
        in_maps.append(m)
    return in_maps


def kernel(adj, features, candidate, graph_pool, actions, action_masks,
           gnn_params, critic_params, value_params, _trace=False):
    nc = _get_nc()
    in_maps = _prep_inputs(adj, features, candidate, graph_pool, actions,
                           action_masks, gnn_params, critic_params, value_params)
    res = run_bass_kernel_spmd(nc, in_maps, list(range(N_CORES)), trace=_trace)
    out = np.concatenate([np.asarray(res.results[c]["qout"][0]) for c in range(N_CORES)])
    if _trace:
        _CACHE["last_results"] = res
    return out.astype(np.float32)


if __name__ == "__main__":
    nc = _get_nc()
    print("build + compile OK")


# revision 39
# speedup vs baseline: 1.0299x; 1.0194x over previous
"""Trainium2 Bass kernel for nn_DiscreteFullyConnectedQFunction (GIN message passing + dueling Q heads).

Strategy: data-parallel over batch (8 graphs per core, 8 cores). All heavy
matmuls in fp16 at full PE rate (T-layout, feature dim on partitions, adjT
streamed as the moving operand); the full 16MB fp16 adjT stays SBUF-resident
so it is read from HBM exactly once, prefetched under the early phases.
Layer-1 pooling (DIN=2, 1.6% of FLOPs) is done on host in fp32 so the adjT
load never gates the first BN sync. Exact training-mode BatchNorm via four
tiny cross-core AllGathers of per-feature (sum, sumsq) — the [128,2] payload
is byte-reinterpreted as [16,16] so the 8-rank concat fits 128 partitions,
then reduced locally (AG has ~2x lower latency than AllReduce). Candidate
gather and graph pooling are folded into one matmul against a host-built
one-hot/pool matrix. Linear biases inside the GIN blocks cancel exactly
under BatchNorm and are dropped. Heads run in fp16 with fp32 PSUM
accumulation and fp32 dueling combine.
"""
import numpy as np
from contextlib import ExitStack

import concourse.bass as bass
import concourse.bacc as bacc
import concourse.tile as tile
import concourse.mybir as mybir
from concourse.bass_utils import run_bass_kernel_spmd
from concourse.masks import make_identity

N_CORES = 8
B, NN, NJ = 64, 1000, 50
GPC = B // N_CORES          # graphs per core
H, HC = 128, 256
NT = 8                      # node tiles of 128 (1000 padded to 1024)
NPAD = NT * 128
BN_EPS = 1e-5
ROWS_CORE = float(GPC * NN)  # BN rows per core
ROWS_TOT = float(B * NN)     # BN rows globally
GCOLS = 52                  # 50 candidate one-hot cols + graph_pool + pad
CH = [(0, 512), (512, 488)]  # free-dim chunks, aligned to 2KB PSUM banks
SLAB = 4                    # adjT m-tiles per DMA
RES = 8                     # graphs whose adjT stays SBUF-resident for pass 2
F16 = mybir.dt.float16
F32 = mybir.dt.float32
AF = mybir.ActivationFunctionType
ALU = mybir.AluOpType
RG = [list(range(N_CORES))]

_CACHE = {}


def _build(debug_dump=False):
    nc = bacc.Bacc("TRN2", target_bir_lowering=False, debug=False, num_devices=N_CORES)
    dbg = {}
    if debug_dump:
        for nm, shape, dt in [
            ("dbg_p1", [2, NN], F16), ("dbg_a1", [H, 1], F32), ("dbg_c1", [H, 1], F32),
            ("dbg_y1", [H, NN], F16), ("dbg_h1nm", [128, NT * 128], F16),
            ("dbg_x2", [H, NN], F16), ("dbg_y2", [H, NN], F16),
            ("dbg_obs", [128, GPC * NJ], F32), ("dbg_hp", [128, GPC], F32),
            ("dbg_qrow", [1, GPC * NJ], F32), ("dbg_vrow", [1, GPC], F32),
            ("dbg_sqm", [1, GPC], F32), ("dbg_qsel", [1, GPC], F32),
        ]:
            dbg[nm] = nc.declare_dram_parameter(nm, shape, dt, isOutput=True)

    adjT = nc.declare_dram_parameter("adjT", [GPC, NPAD, NN], F16, isOutput=False)
    p1t = nc.declare_dram_parameter("p1t", [2, 2, GPC // 2 * NN], F16, isOutput=False)
    gmat = nc.declare_dram_parameter("gmat", [GPC, NT, 128, GCOLS], F16, isOutput=False)
    w1 = nc.declare_dram_parameter("w1", [2, H], F16, isOutput=False)
    w2 = nc.declare_dram_parameter("w2", [H, H], F16, isOutput=False)
    w1p = nc.declare_dram_parameter("w1p", [H, H], F16, isOutput=False)
    w2p = nc.declare_dram_parameter("w2p", [H, H], F16, isOutput=False)
    vecs1 = nc.declare_dram_parameter("vecs1", [H, 4], F32, isOutput=False)  # g_in,b_in,g_out,b_out
    vecs2 = nc.declare_dram_parameter("vecs2", [H, 4], F32, isOutput=False)
    wc1 = nc.declare_dram_parameter("wc1", [2 * H, HC], F16, isOutput=False)
    wc2 = nc.declare_dram_parameter("wc2", [HC, HC], F16, isOutput=False)
    wc3 = nc.declare_dram_parameter("wc3", [HC, 1], F16, isOutput=False)
    bc = nc.declare_dram_parameter("bc", [H, 4], F32, isOutput=False)  # b1c lo/hi, b2c lo/hi
    wv1 = nc.declare_dram_parameter("wv1", [H, HC], F16, isOutput=False)
    wv2 = nc.declare_dram_parameter("wv2", [HC, HC], F16, isOutput=False)
    wv3 = nc.declare_dram_parameter("wv3", [HC, 1], F16, isOutput=False)
    bv = nc.declare_dram_parameter("bv", [H, 4], F32, isOutput=False)
    maskr = nc.declare_dram_parameter("maskr", [1, GPC * NJ], F32, isOutput=False)
    ohr = nc.declare_dram_parameter("ohr", [1, GPC * NJ], F32, isOutput=False)
    minv = nc.declare_dram_parameter("minv", [1, GPC], F32, isOutput=False)
    scal = nc.declare_dram_parameter("scal", [1, 4], F32, isOutput=False)  # [b3v, ...]
    qout = nc.declare_dram_parameter("qout", [1, GPC], F32, isOutput=True)

    with tile.TileContext(nc) as tc, ExitStack() as ctx:
        const = ctx.enter_context(tc.tile_pool(name="const", bufs=1))
        small = ctx.enter_context(tc.tile_pool(name="small", bufs=1))
        dram = ctx.enter_context(tc.tile_pool(name="dram", bufs=1, space="DRAM"))
        statsp = ctx.enter_context(tc.tile_pool(name="stats", bufs=1))
        # one psum pool for the whole kernel: "mm" [128,1000] f32 x3 (6 banks)
        # + "tp" [128,128] f16 x2 (2 banks) = 8 banks
        psum = ctx.enter_context(tc.tile_pool(name="psum", bufs=1, space="PSUM"))

        # p1 (host-computed layer-1 pooling) loads FIRST on the sync queue so
        # phase A's matmuls are not stuck behind the constant-parameter DMAs
        p1sb_p = ctx.enter_context(tc.tile_pool(name="p1sb", bufs=2))
        p1half = []
        for hh in range(2):
            p1h = p1sb_p.tile([2, GPC // 2 * NN], F16, tag="p1sb", name="p1sb")
            nc.sync.dma_start(out=p1h[:], in_=p1t[hh])
            p1half.append(p1h)

        # ---- constants / params in SBUF
        ident16 = const.tile([128, 128], F16, tag="ident")
        make_identity(nc, ident16[:])
        eps_t = const.tile([128, 1], F32, tag="eps")
        nc.vector.memset(eps_t[:], BN_EPS)

        def _load(name, ap, shape, dt, tag):
            t = const.tile(shape, dt, tag=tag, name=tag)
            nc.sync.dma_start(out=t[:], in_=ap)
            return t

        w1_sb = _load("w1", w1[:], [2, H], F16, "w1")
        w2_sb = _load("w2", w2[:], [H, H], F16, "w2")
        w1p_sb = _load("w1p", w1p[:], [H, H], F16, "w1p")
        w2p_sb = _load("w2p", w2p[:], [H, H], F16, "w2p")
        vecs1_sb = _load("vecs1", vecs1[:], [H, 4], F32, "vecs1")
        vecs2_sb = _load("vecs2", vecs2[:], [H, 4], F32, "vecs2")
        bc_sb = _load("bc", bc[:], [H, 4], F32, "bc")
        bv_sb = _load("bv", bv[:], [H, 4], F32, "bv")
        mask_sb = _load("maskr", maskr[:], [1, GPC * NJ], F32, "maskr")
        oh_sb = _load("ohr", ohr[:], [1, GPC * NJ], F32, "ohr")
        minv_sb = _load("minv", minv[:], [1, GPC], F32, "minv")
        scal_sb = _load("scal", scal[:], [1, 4], F32, "scal")


        stats_t = [statsp.tile([H, 2 * GPC, 6], F32, tag=f"stats{i}", name=f"stats{i}")
                   for i in range(4)]

        # ---- persistent activation pools
        y1sb_p = ctx.enter_context(tc.tile_pool(name="y1sb", bufs=GPC))
        h1nm_p = ctx.enter_context(tc.tile_pool(name="h1nm", bufs=2))
        sync2_p = ctx.enter_context(tc.tile_pool(name="sync2", bufs=GPC + 1))
        trans_p = ctx.enter_context(tc.tile_pool(name="trans", bufs=2))
        headp = ctx.enter_context(tc.tile_pool(name="headp", bufs=1))

        obs_top = headp.tile([128, GPC * NJ], F16, tag="obs_top")
        hp_all = headp.tile([128, GPC], F16, tag="hp_all")

        p1sb, y1sb, h1nm_g, x2sb, y2sb = {}, {}, {}, {}, {}

        def mm_tile():
            return psum.tile([128, 1000], F32, tag="mm", name="mm", bufs=3)

        def tp_tile():
            return psum.tile([128, NT * 128], F16, tag="tp", name="tp", bufs=2)

        adj_res = {}

        def load_adj_slabs(g, keep=True):
            """Strided DMAs of [128, SLAB, 1000] covering the graph's adjT."""
            slabs = []
            for s in range(NT // SLAB):
                sl = adj_res_p.tile([128, SLAB, NN], F16, tag="adjr", name="adjr")
                src = adjT[g, s * SLAB * 128:(s + 1) * SLAB * 128, :]
                nc.sync.dma_start(out=sl[:], in_=src.rearrange("(u p) n -> p u n", p=128))
                slabs.append(sl)
            adj_res[g] = slabs
            return slabs

        def adj_rhs(slabs, t, c0, cw):
            return slabs[t // SLAB][:, t % SLAB, c0:c0 + cw]

        def bn_sync(idx, gamma_col, beta_col):
            """AllReduce per-feature stats -> per-partition affine (a, c)."""
            mv = small.tile([H, 2], F32, tag=f"mv{idx}", name=f"mv{idx}")
            nc.vector.bn_aggr(out=mv[:], in_=stats_t[idx][:])
            ssum = small.tile([H, 2], F32, tag=f"ssum{idx}", name=f"ssum{idx}")
            msq = small.tile([H, 1], F32, tag=f"msq{idx}", name=f"msq{idx}")
            nc.vector.tensor_mul(out=msq[:], in0=mv[:, 0:1], in1=mv[:, 0:1])
            nc.vector.tensor_add(out=ssum[:, 1:2], in0=mv[:, 1:2], in1=msq[:])
            nc.vector.tensor_scalar_mul(out=ssum[:, 1:2], in0=ssum[:, 1:2], scalar1=ROWS_CORE)
            nc.vector.tensor_scalar_mul(out=ssum[:, 0:1], in0=mv[:, 0:1], scalar1=ROWS_CORE)
            cc_in = dram.tile([H, 2], F32, tag=f"ccin{idx}", name=f"ccin{idx}")
            cc_out = dram.tile([N_CORES, H, 2], F32, tag=f"ccout{idx}", name=f"ccout{idx}",
                               addr_space="Shared")
            # gpsimd queue: keeps the tiny sync DMA out of the bulk-DMA SP stream
            nc.gpsimd.dma_start(out=cc_in[:], in_=ssum[:])
            # AllGather has ~2x lower latency than AllReduce at this size; the
            # [128,2] payload is byte-reinterpreted as [16,16] so the gathered
            # output stays within 128 partitions, then reduced locally.
            nc.gpsimd.collective_compute(
                "AllGather", ALU.bypass, replica_groups=RG,
                ins=[cc_in[:].flatten().rearrange("(p c) -> p c", c=16).opt()],
                outs=[cc_out[:].flatten().rearrange("(p c) -> p c", c=16).opt()],
            )
            gl8 = small.tile([H, N_CORES, 2], F32, tag=f"gl8{idx}", name=f"gl8{idx}")
            nc.gpsimd.dma_start(out=gl8[:], in_=cc_out[:].rearrange("r f s -> f r s"))
            gl = small.tile([H, 2], F32, tag=f"gl{idx}", name=f"gl{idx}")
            nc.vector.reduce_sum(out=gl[:], in_=gl8[:].rearrange("f r s -> f s r"),
                                 axis=mybir.AxisListType.X)
            mu = small.tile([H, 1], F32, tag=f"mu{idx}", name=f"mu{idx}")
            var = small.tile([H, 1], F32, tag=f"var{idx}", name=f"var{idx}")
            nc.vector.tensor_scalar_mul(out=mu[:], in0=gl[:, 0:1], scalar1=1.0 / ROWS_TOT)
            nc.vector.tensor_scalar_mul(out=var[:], in0=gl[:, 1:2], scalar1=1.0 / ROWS_TOT)
            nc.vector.tensor_mul(out=msq[:], in0=mu[:], in1=mu[:])
            nc.vector.tensor_sub(out=var[:], in0=var[:], in1=msq[:])
            std = small.tile([H, 1], F32, tag=f"std{idx}", name=f"std{idx}")
            nc.scalar.activation(out=std[:], in_=var[:], func=AF.Sqrt, bias=eps_t[:], scale=1.0)
            rstd = small.tile([H, 1], F32, tag=f"rstd{idx}", name=f"rstd{idx}")
            nc.vector.reciprocal(out=rstd[:], in_=std[:])
            a = small.tile([H, 1], F32, tag=f"a{idx}", name=f"a{idx}")
            c = small.tile([H, 1], F32, tag=f"c{idx}", name=f"c{idx}")
            nc.vector.tensor_mul(out=a[:], in0=rstd[:], in1=gamma_col)
            nc.vector.tensor_mul(out=c[:], in0=a[:], in1=mu[:])
            nc.vector.tensor_sub(out=c[:], in0=beta_col, in1=c[:])
            return a, c

        # ================= Phase A: pool1 + x1 stats =================
        adj_ctx = ExitStack()
        adj_res_p = adj_ctx.enter_context(tc.tile_pool(name="adjres", bufs=RES * (NT // SLAB)))
        work_p = adj_ctx.enter_context(tc.tile_pool(name="workp", bufs=2))
        for g in range(GPC):
            p1sb[g] = p1half[g // 4][:, (g % 4) * NN:(g % 4 + 1) * NN]
            xps = mm_tile()
            for (c0, cw) in CH:
                nc.tensor.matmul(xps[:, c0:c0 + cw], w1_sb[:], p1sb[g][:, c0:c0 + cw],
                                 start=True, stop=True)
            for ci, (c0, cw) in enumerate(CH):
                nc.vector.bn_stats(out=stats_t[0][:, 2 * g + ci, :], in_=xps[:, c0:c0 + cw])

        for g in range(GPC):
            load_adj_slabs(g)  # layer-2 prefetch; overlaps AG1/B/AG2 in the DMA engines

        a1, c1 = bn_sync(0, vecs1_sb[:, 0:1], vecs1_sb[:, 1:2])

        # ================= Phase B: z1 -> y1 + stats =================
        prev_drain = None
        for g in range(GPC):
            xps = mm_tile()
            for (c0, cw) in CH:
                nc.tensor.matmul(xps[:, c0:c0 + cw], w1_sb[:], p1sb[g][:, c0:c0 + cw],
                                 start=True, stop=True)
            z1 = work_p.tile([128, NN], F16, tag="z1", name="z1")
            nc.scalar.activation(out=z1[:], in_=xps[:], func=AF.Relu, bias=c1[:], scale=a1[:])
            yps = mm_tile()
            for (c0, cw) in CH:
                nc.tensor.matmul(yps[:, c0:c0 + cw], w2_sb[:], z1[:, c0:c0 + cw],
                                 start=True, stop=True)
            for ci, (c0, cw) in enumerate(CH):
                nc.vector.bn_stats(out=stats_t[1][:, 2 * g + ci, :], in_=yps[:, c0:c0 + cw])
            y1 = y1sb_p.tile([128, NN], F16, tag="y1sb", name="y1sb")
            y1sb[g] = y1
            # drain is software-pipelined one graph behind so ACT never waits on PE
            if prev_drain is not None:
                nc.scalar.activation(out=prev_drain[1][:], in_=prev_drain[0][:], func=AF.Copy)
            prev_drain = (yps, y1)
        nc.scalar.activation(out=prev_drain[1][:], in_=prev_drain[0][:], func=AF.Copy)

        a2, c2 = bn_sync(1, vecs1_sb[:, 2:3], vecs1_sb[:, 3:4])

        # ===== Phase C: h1 = relu(BN(y1)), transpose, pool2, x2 stats =====
        for g in range(GPC):
            h1t = trans_p.tile([128, NPAD], F16, tag="h1t", name="h1t")
            nc.scalar.activation(out=h1t[:, 0:NN], in_=y1sb[g][:], func=AF.Relu,
                                 bias=c2[:], scale=a2[:])
            nc.vector.memset(h1t[:, NN:NPAD], 0.0)
            h1nm = h1nm_p.tile([128, NT * 128], F16, tag="h1nm", name="h1nm")
            h1nm_g[g] = h1nm
            tp = tp_tile()
            for t in range(NT):
                nc.tensor.transpose(tp[:, t * 128:(t + 1) * 128],
                                    h1t[:, t * 128:(t + 1) * 128], ident16[:])
            half = NT * 64
            nc.vector.tensor_copy(out=h1nm[:, 0:half], in_=tp[:, 0:half])
            nc.scalar.activation(out=h1nm[:, half:], in_=tp[:, half:], func=AF.Copy)
            slabs = adj_res[g]
            p2ps = mm_tile()
            for t in range(NT):
                for (c0, cw) in CH:
                    nc.tensor.matmul(p2ps[:, c0:c0 + cw], h1nm[:, t * 128:(t + 1) * 128],
                                     adj_rhs(slabs, t, c0, cw), start=(t == 0), stop=(t == NT - 1))
            p2 = work_p.tile([128, NN], F16, tag="p2sb", name="p2sb")
            nc.scalar.activation(out=p2[:], in_=p2ps[:], func=AF.Copy)
            x2ps = mm_tile()
            for (c0, cw) in CH:
                nc.tensor.matmul(x2ps[:, c0:c0 + cw], w1p_sb[:], p2[:, c0:c0 + cw],
                                 start=True, stop=True)
            for ci, (c0, cw) in enumerate(CH):
                nc.vector.bn_stats(out=stats_t[2][:, 2 * g + ci, :], in_=x2ps[:, c0:c0 + cw])
            x2 = sync2_p.tile([128, NN], F16, tag="s2", name="x2sb")
            x2sb[g] = x2
            nc.vector.tensor_copy(out=x2[:], in_=x2ps[:])

        adj_ctx.close()
        late_p = ctx.enter_context(tc.tile_pool(name="latep", bufs=2))
        gm_p = ctx.enter_context(tc.tile_pool(name="gmp", bufs=3))

        a3, c3 = bn_sync(2, vecs2_sb[:, 0:1], vecs2_sb[:, 1:2])

        # ================= Phase D: z2 -> y2 + stats =================
        prev_drain = None
        for g in range(GPC):
            z2 = late_p.tile([128, NN], F16, tag="z2", name="z2")
            nc.scalar.activation(out=z2[:], in_=x2sb[g][:], func=AF.Relu, bias=c3[:], scale=a3[:])
            yps = mm_tile()
            for (c0, cw) in CH:
                nc.tensor.matmul(yps[:, c0:c0 + cw], w2p_sb[:], z2[:, c0:c0 + cw],
                                 start=True, stop=True)
            for ci, (c0, cw) in enumerate(CH):
                nc.vector.bn_stats(out=stats_t[3][:, 2 * g + ci, :], in_=yps[:, c0:c0 + cw])
            y2 = sync2_p.tile([128, NN], F16, tag="s2", name="y2sb")
            y2sb[g] = y2
            if prev_drain is not None:
                nc.scalar.activation(out=prev_drain[1][:], in_=prev_drain[0][:], func=AF.Copy)
            prev_drain = (yps, y2)
        nc.scalar.activation(out=prev_drain[1][:], in_=prev_drain[0][:], func=AF.Copy)

        a4, c4 = bn_sync(3, vecs2_sb[:, 2:3], vecs2_sb[:, 3:4])

        # ===== Phase E: h2, transpose, gather (candidates + graph pool) =====
        for g in range(GPC):
            h2t = trans_p.tile([128, NPAD], F16, tag="h2t", name="h2t")
            nc.scalar.activation(out=h2t[:, 0:NN], in_=y2sb[g][:], func=AF.Relu,
                                 bias=c4[:], scale=a4[:])
            nc.vector.memset(h2t[:, NN:NPAD], 0.0)
            h2nm = late_p.tile([128, NT * 128], F16, tag="h2nm", name="h2nm")
            tp = tp_tile()
            for t in range(NT):
                nc.tensor.transpose(tp[:, t * 128:(t + 1) * 128],
                                    h2t[:, t * 128:(t + 1) * 128], ident16[:])
            half = NT * 64
            nc.vector.tensor_copy(out=h2nm[:, 0:half], in_=tp[:, 0:half])
            nc.scalar.activation(out=h2nm[:, half:], in_=tp[:, half:], func=AF.Copy)
            gm = gm_p.tile([128, NT, GCOLS], F16, tag="gm", name="gm")
            nc.sync.dma_start(out=gm[:], in_=gmat[g].rearrange("t p c -> p t c"))
            gps = mm_tile()
            for t in range(NT):
                nc.tensor.matmul(gps[:, 0:GCOLS], h2nm[:, t * 128:(t + 1) * 128], gm[:, t, :],
                                 start=(t == 0), stop=(t == NT - 1))
            nc.vector.tensor_copy(out=obs_top[:, g * NJ:(g + 1) * NJ], in_=gps[:, 0:NJ])
            nc.vector.tensor_copy(out=hp_all[:, g:g + 1], in_=gps[:, NJ:NJ + 1])

        # ================= Phase F: heads + dueling =================
        with tc.tile_pool(name="hf", bufs=1) as hf:
            def _loadh(ap, shape, tag):
                t = hf.tile(shape, F16, tag=tag, name=tag)
                nc.sync.dma_start(out=t[:], in_=ap)
                return t

            wc1_t, wc2_t, wv2_t = {}, {}, {}
            for kh in range(2):
                for oh in range(2):
                    wc1_t[(kh, oh)] = _loadh(wc1[kh * 128:(kh + 1) * 128, oh * 128:(oh + 1) * 128],
                                             [128, 128], f"wc1_{kh}{oh}")
                    wc2_t[(kh, oh)] = _loadh(wc2[kh * 128:(kh + 1) * 128, oh * 128:(oh + 1) * 128],
                                             [128, 128], f"wc2_{kh}{oh}")
                    wv2_t[(kh, oh)] = _loadh(wv2[kh * 128:(kh + 1) * 128, oh * 128:(oh + 1) * 128],
                                             [128, 128], f"wv2_{kh}{oh}")
            wv1_t = [_loadh(wv1[:, oh * 128:(oh + 1) * 128], [128, 128], f"wv1_{oh}")
                     for oh in range(2)]
            wc3_h = [_loadh(wc3[kh * 128:(kh + 1) * 128, :], [128, 1], f"wc3_{kh}")
                     for kh in range(2)]
            wv3_h = [_loadh(wv3[kh * 128:(kh + 1) * 128, :], [128, 1], f"wv3_{kh}")
                     for kh in range(2)]
            # critic bias path: vbb[oh] = W1c_bot.T @ h_pooled + b1c
            vbb = []
            for oh in range(2):
                ps = mm_tile()
                nc.tensor.matmul(ps[0:128, 0:GPC], wc1_t[(1, oh)][:], hp_all[:], start=True, stop=True)
                vb = hf.tile([128, GPC], F32, tag=f"vbb{oh}", name=f"vbb{oh}")
                nc.vector.tensor_scalar_add(out=vb[:], in0=ps[0:128, 0:GPC], scalar1=bc_sb[:, oh:oh + 1])
                vbb.append(vb)
            # critic L1
            a1c = []
            for oh in range(2):
                ps = mm_tile()
                nc.tensor.matmul(ps[:, 0:GPC * NJ], wc1_t[(0, oh)][:], obs_top[:], start=True, stop=True)
                acts = hf.tile([128, GPC * NJ], F16, tag=f"a1c{oh}", name=f"a1c{oh}")
                for g in range(GPC):
                    nc.scalar.activation(out=acts[:, g * NJ:(g + 1) * NJ],
                                         in_=ps[:, g * NJ:(g + 1) * NJ],
                                         func=AF.Relu, bias=vbb[oh][:, g:g + 1], scale=1.0)
                a1c.append(acts)
            # critic L2
            a2c = []
            for oh in range(2):
                ps = mm_tile()
                nc.tensor.matmul(ps[:, 0:GPC * NJ], wc2_t[(0, oh)][:], a1c[0][:], start=True, stop=False)
                nc.tensor.matmul(ps[:, 0:GPC * NJ], wc2_t[(1, oh)][:], a1c[1][:], start=False, stop=True)
                acts = hf.tile([128, GPC * NJ], F16, tag=f"a2c{oh}", name=f"a2c{oh}")
                nc.scalar.activation(out=acts[:], in_=ps[:, 0:GPC * NJ], func=AF.Relu,
                                     bias=bc_sb[:, 2 + oh:3 + oh], scale=1.0)
                a2c.append(acts)
            # critic L3 -> q_row [1, 400]
            psq3 = mm_tile()
            nc.tensor.matmul(psq3[0:1, 0:GPC * NJ], wc3_h[0][:], a2c[0][:], start=True, stop=False)
            nc.tensor.matmul(psq3[0:1, 0:GPC * NJ], wc3_h[1][:], a2c[1][:], start=False, stop=True)
            qrow = hf.tile([1, GPC * NJ], F32, tag="qrow", name="qrow")
            nc.vector.tensor_copy(out=qrow[:], in_=psq3[0:1, 0:GPC * NJ])
            # value head
            a1v = []
            for oh in range(2):
                ps = mm_tile()
                nc.tensor.matmul(ps[0:128, 0:GPC], wv1_t[oh][:], hp_all[:], start=True, stop=True)
                acts = hf.tile([128, GPC], F16, tag=f"a1v{oh}", name=f"a1v{oh}")
                nc.scalar.activation(out=acts[:], in_=ps[0:128, 0:GPC], func=AF.Relu,
                                     bias=bv_sb[:, oh:oh + 1], scale=1.0)
                a1v.append(acts)
            a2v = []
            for oh in range(2):
                ps = mm_tile()
                nc.tensor.matmul(ps[0:128, 0:GPC], wv2_t[(0, oh)][:], a1v[0][:], start=True, stop=False)
                nc.tensor.matmul(ps[0:128, 0:GPC], wv2_t[(1, oh)][:], a1v[1][:], start=False, stop=True)
                acts = hf.tile([128, GPC], F16, tag=f"a2v{oh}", name=f"a2v{oh}")
                nc.scalar.activation(out=acts[:], in_=ps[0:128, 0:GPC], func=AF.Relu,
                                     bias=bv_sb[:, 2 + oh:3 + oh], scale=1.0)
                a2v.append(acts)
            psvf = mm_tile()
            nc.tensor.matmul(psvf[0:1, 0:GPC], wv3_h[0][:], a2v[0][:], start=True, stop=False)
            nc.tensor.matmul(psvf[0:1, 0:GPC], wv3_h[1][:], a2v[1][:], start=False, stop=True)
            vrow = hf.tile([1, GPC], F32, tag="vrow", name="vrow")
            nc.vector.tensor_copy(out=vrow[:], in_=psvf[0:1, 0:GPC])
            # dueling combine: out = v + b3v + q[action] - sum(q*mask)/sum(mask)
            qm = hf.tile([1, GPC * NJ], F32, tag="qm", name="qm")
            nc.vector.tensor_mul(out=qm[:], in0=qrow[:], in1=mask_sb[:])
            sqm = hf.tile([1, GPC], F32, tag="sqm", name="sqm")
            nc.vector.reduce_sum(out=sqm[:].unsqueeze(2),
                                 in_=qm[:].rearrange("p (g j) -> p g j", g=GPC),
                                 axis=mybir.AxisListType.X)
            masked = hf.tile([1, GPC], F32, tag="masked", name="masked")
            nc.vector.tensor_mul(out=masked[:], in0=sqm[:], in1=minv_sb[:])
            qs = hf.tile([1, GPC * NJ], F32, tag="qs", name="qs")
            nc.vector.tensor_mul(out=qs[:], in0=qrow[:], in1=oh_sb[:])
            qsel = hf.tile([1, GPC], F32, tag="qsel", name="qsel")
            nc.vector.reduce_sum(out=qsel[:].unsqueeze(2),
                                 in_=qs[:].rearrange("p (g j) -> p g j", g=GPC),
                                 axis=mybir.AxisListType.X)
            qf = hf.tile([1, GPC], F32, tag="qf", name="qf")
            nc.vector.tensor_add(out=qf[:], in0=vrow[:], in1=qsel[:])
            nc.vector.tensor_sub(out=qf[:], in0=qf[:], in1=masked[:])
            nc.vector.tensor_scalar_add(out=qf[:], in0=qf[:], scalar1=scal_sb[0:1, 0:1])
            nc.sync.dma_start(out=qout[:], in_=qf[:])
            if debug_dump:
                nc.sync.dma_start(out=dbg["dbg_p1"][:], in_=p1sb[0][:])
                nc.sync.dma_start(out=dbg["dbg_a1"][:], in_=a1[:])
                nc.sync.dma_start(out=dbg["dbg_c1"][:], in_=c1[:])
                nc.sync.dma_start(out=dbg["dbg_y1"][:], in_=y1sb[0][:])
                nc.sync.dma_start(out=dbg["dbg_h1nm"][:], in_=h1nm_g[0][:])
                nc.sync.dma_start(out=dbg["dbg_x2"][:], in_=x2sb[0][:])
                nc.sync.dma_start(out=dbg["dbg_y2"][:], in_=y2sb[0][:])
                nc.sync.dma_start(out=dbg["dbg_obs"][:], in_=obs_top[:])
                nc.sync.dma_start(out=dbg["dbg_hp"][:], in_=hp_all[:])
                nc.sync.dma_start(out=dbg["dbg_qrow"][:], in_=qrow[:])
                nc.sync.dma_start(out=dbg["dbg_vrow"][:], in_=vrow[:])
                nc.sync.dma_start(out=dbg["dbg_sqm"][:], in_=sqm[:])
                nc.sync.dma_start(out=dbg["dbg_qsel"][:], in_=qsel[:])

    nc.compile()
    return nc


def _get_nc():
    if "nc" not in _CACHE:
        _CACHE["nc"] = _build()
    return _CACHE["nc"]


def _prep_inputs(adj, features, candidate, graph_pool, actions, action_masks,
                 gnn_params, critic_params, value_params):
    adj = np.asarray(adj, dtype=np.float32)
    features = np.asarray(features, dtype=np.float32)
    candidate = np.asarray(candidate).astype(np.int64)
    graph_pool = np.asarray(graph_pool, dtype=np.float32)
    actions = np.asarray(actions).astype(np.int64)
    mask = np.asarray(action_masks).astype(np.float32)

    adjTp = np.zeros((B, NPAD, NN), np.float16)
    adjTp[:, :NN, :] = adj.transpose(0, 2, 1)
    pooled1 = np.matmul(adj, features)               # [B, N, 2] fp32 (layer-1 message passing)
    p1tp = pooled1.transpose(0, 2, 1).astype(np.float16)         # [B, 2, N]
    # per-core packing: [2 halves, 2 features, 4*N] with graphs concatenated on the free axis
    p1pack = p1tp.reshape(N_CORES, 2, GPC // 2, 2, NN).transpose(0, 1, 3, 2, 4) \
                 .reshape(N_CORES, 2, 2, GPC // 2 * NN)
    gmatp = np.zeros((B, NPAD, GCOLS), np.float16)
    gmatp[np.arange(B)[:, None], candidate, np.arange(NJ)[None, :]] = 1.0
    gmatp[:, :NN, NJ] = graph_pool
    gmatp = gmatp.reshape(B, NT, 128, GCOLS)

    p1, p2 = gnn_params
    def f32(x):
        return np.ascontiguousarray(np.asarray(x, dtype=np.float32))
    def f16(x):
        return np.ascontiguousarray(np.asarray(x, dtype=np.float16))

    w1 = f16(p1["W1"]); w2 = f16(p1["W2"])
    w1p = f16(p2["W1"]); w2p = f16(p2["W2"])
    vecs1 = np.stack([f32(p1["bn1_g"]), f32(p1["bn1_b"]),
                      f32(p1["bn_g"]), f32(p1["bn_b"])], axis=1)
    vecs2 = np.stack([f32(p2["bn1_g"]), f32(p2["bn1_b"]),
                      f32(p2["bn_g"]), f32(p2["bn_b"])], axis=1)
    wc1 = f16(critic_params["W1"]); wc2 = f16(critic_params["W2"]); wc3 = f16(critic_params["W3"])
    b1c = f32(critic_params["b1"]); b2c = f32(critic_params["b2"])
    b3c = float(np.asarray(critic_params["b3"]).reshape(-1)[0])  # cancels in dueling
    bcm = np.stack([b1c[:128], b1c[128:], b2c[:128], b2c[128:]], axis=1)
    wv1 = f16(value_params["W1"]); wv2 = f16(value_params["W2"]); wv3 = f16(value_params["W3"])
    b1v = f32(value_params["b1"]); b2v = f32(value_params["b2"])
    b3v = float(np.asarray(value_params["b3"]).reshape(-1)[0])
    bvm = np.stack([b1v[:128], b1v[128:], b2v[:128], b2v[128:]], axis=1)

    onehot = np.zeros((B, NJ), np.float32)
    onehot[np.arange(B), actions[:, 0]] = 1.0
    minv = (1.0 / np.maximum(mask.sum(axis=1), 1e-9)).astype(np.float32)
    scal = np.array([[b3v, b3c, 0.0, 0.0]], np.float32)

    shared = {
        "w1": w1, "w2": w2, "w1p": w1p, "w2p": w2p,
        "vecs1": np.ascontiguousarray(vecs1, np.float32),
        "vecs2": np.ascontiguousarray(vecs2, np.float32),
        "wc1": wc1, "wc2": wc2, "wc3": wc3.reshape(HC, 1),
        "bc": np.ascontiguousarray(bcm, np.float32),
        "wv1": wv1, "wv2": wv2, "wv3": wv3.reshape(HC, 1),
        "bv": np.ascontiguousarray(bvm, np.float32),
        "scal": scal,
    }
    in_maps = []
    for c in range(N_CORES):
        s = slice(c * GPC, (c + 1) * GPC)
        m = dict(shared)
        m["adjT"] = np.ascontiguousarray(adjTp[s])
        m["p1t"] = np.ascontiguousarray(p1pack[c])
        m["gmat"] = np.ascontiguousarray(gmatp[s])
        m["maskr"] = np.ascontiguousarray(mask[s].reshape(1, GPC * NJ))
        m["ohr"] = np.ascontiguousarray(onehot[s].reshape(1, GPC * NJ))
        m["minv"] = np.ascontiguousarray(minv[s].reshape(1, GPC))
        in_maps.append(m)
    return in_maps


def kernel(adj, features, candidate, graph_pool, actions, action_masks,
           gnn_params, critic_params, value_params, _trace=False):
    nc = _get_nc()
    in_maps = _prep_inputs(adj, features, candidate, graph_pool, actions,
                           action_masks, gnn_params, critic_params, value_params)
    res = run_bass_kernel_spmd(nc, in_maps, list(range(N_CORES)), trace=_trace)
    out = np.concatenate([np.asarray(res.results[c]["qout"][0]) for c in range(N_CORES)])
    if _trace:
        _CACHE["last_results"] = res
    return out.astype(np.float32)


if __name__ == "__main__":
    nc = _get_nc()
    print("build + compile OK")
